# revision 10
# baseline (speedup 1.0000x reference)
"""Trainium2 Bass kernel for nn_CEINN_67138928771458.

Data-parallel over batch across 8 NeuronCores (8 batches/core), params
replicated. Activations kept token-major ([128 tokens, feat] tiles); PE
transposes produce feature-major matmul operands. Matmuls run in float32r
(full-rate fp32 with mantissa rounding). Attention uses the identity
softmax(s + hyper) = exp(s)*exp(hyper) / sum, with exp(hyper) =
1/(1 + gamma*delta + 1e-8) precomputed per batch (masked entries get
delta=1e35 so the weight underflows to ~0).
"""
import sys

sys.path.insert(0, "/opt/trn_rl_repo")

import numpy as np
import concourse.bass as bass
import concourse.tile as tile
from concourse import mybir, bacc
from concourse.bass_utils import run_bass_kernel_spmd
from concourse.masks import make_identity

f32 = mybir.dt.float32
f32r = mybir.dt.float32r
i32 = mybir.dt.int32
AF = mybir.ActivationFunctionType
ALU = mybir.AluOpType
AX = mybir.AxisListType

B, S, C, D, H, L = 64, 512, 200, 256, 4, 2
SEM, ECO, P, NI, SIDE, DH = 128, 128, 8, 50000, 64, 64
ALPHA, BETA, LAM, GTEMP, TAU = 0.7, 0.8, 2.0, 0.5, 10.0
NCORE, BL = 8, 8
T = BL * S
NT = T // 128
CP = 256
TCD = BL * CP
NCT = TCD // 128

_FMB_COLS = {}
_c = 0
for _nm, _n in [("qb0", 2), ("qb1", 2), ("kb0", 2), ("kb1", 2),
                ("ecob1", 2), ("ecob2", 1), ("semb1", 2), ("semb2", 1),
                ("ffb1_0", 8), ("ffb1_1", 8), ("dob", 2), ("shb1", 2),
                ("sideb", 2)]:
    _FMB_COLS[_nm] = _c
    _c += _n
NFMB = _c

_BCC = {}
for _i, _nm in enumerate(["sideb", "vb0", "vb1", "ob0", "ob1", "ffb2_0",
                          "ffb2_1", "n1g0", "n1b0", "n2g0", "n2b0",
                          "n1g1", "n1b1", "n2g1", "n2b1", "fng", "fnb"]):
    _BCC[_nm] = _i
NBCC = len(_BCC)

_SWV = {"gamW": 0, "refW": 1, "etaW": 2, "shW2_0": 3, "shW2_1": 4,
        "doOutW_0": 5, "doOutW_1": 6, "ones": 7}
NSWV = 8
# scal cols: 0=gamb 1=refb 2=etab 3=1e-5 4=1e-6 5=1+1e-8 6=doOutb 7=shb2
#            8=etaW_last 9=kscale
NSCAL = 10

_PROG_CACHE = {}


def _build_program(dbg=False, zflags=frozenset()):
    nc = bacc.Bacc(None, target_bir_lowering=False)

    def din(name, shape, dtype=f32):
        return nc.declare_dram_parameter(name, list(shape), dtype, isOutput=False)

    def dout(name, shape, dtype=f32):
        return nc.declare_dram_parameter(name, list(shape), dtype, isOutput=True)

    item_tab = din("item_tab", [NI + 1, D])
    side_tab = din("side_tab", [NI + 1, SIDE])
    prop_tab = din("prop_tab", [NI + 1, 1])
    seq_idx = din("seq_idx", [128, NT], i32)
    cand_idx = din("cand_idx", [128, NCT], i32)
    last_idx = din("last_idx", [8, 1], i32)
    pos4 = din("pos4", [4, 128, D])
    delta4 = din("delta4", [4, 128, S])
    qw = din("qw", [L, 2, 128, D]); kw = din("kw", [L, 2, 128, D])
    vw = din("vw", [L, 2, 128, D]); ow = din("ow", [L, 2, 128, D])
    f1w = din("f1w", [L, 2, 128, 4 * D]); f2w = din("f2w", [L, 8, 128, D])
    ecw1 = din("ecw1", [2, 128, D]); ecw2 = din("ecw2", [2, 128, ECO])
    smw1 = din("smw1", [2, 128, D]); smw2 = din("smw2", [2, 128, SEM])
    sdw = din("sdw", [SIDE, D])
    wiw = din("wiw", [2, 128, D]); wew = din("wew", [128, D])
    wi2w = din("wi2w", [2, 128, D]); wsw = din("wsw", [128, D])
    swv = din("swv", [128, NSWV])
    fmb = din("fmb", [128, NFMB])
    bcc = din("bcc", [NBCC, 128, D])
    prior_bc = din("prior_bc", [8, CP, P])
    pwp = din("pwp", [128, 2, P])
    scal = din("scal", [128, NSCAL])

    y_total = dout("y_total", [BL, C]); y_short = dout("y_short", [BL, C])
    y_long = dout("y_long", [BL, C]); y_do = dout("y_do", [BL, C])
    y_ref = dout("y_ref", [BL]); y_w = dout("y_w", [BL, C])
    y_osq = dout("y_osq", [BL])
    if dbg:
        y_dx0 = dout("y_dx0", [128, D])
        y_dgam = dout("y_dgam", [1, T])
        y_dl0 = dout("y_dl0", [128, D])
        y_dxf = dout("y_dxf", [128, D])
        y_decs = dout("y_decs", [128, 8])
        y_dattn = dout("y_dattn", [128, S])

    with tile.TileContext(nc) as tc:
        with tc.tile_pool(name="cpool", bufs=1) as cpool, \
             tc.tile_pool(name="dram", bufs=1, space="DRAM") as dram:

            ident = cpool.tile([128, 128], f32)
            make_identity(nc, ident)

            pos_sb = cpool.tile([128, 4, D], f32)
            for j in range(4):
                nc.sync.dma_start(out=pos_sb[:, j, :], in_=pos4[j])
            bcc_sb = cpool.tile([128, NBCC, D], f32)
            for j in range(NBCC):
                nc.sync.dma_start(out=bcc_sb[:, j, :], in_=bcc[j])
            fmb_sb = cpool.tile([128, NFMB], f32)
            nc.sync.dma_start(out=fmb_sb[:], in_=fmb[:])
            scal_sb = cpool.tile([128, NSCAL], f32)
            nc.sync.dma_start(out=scal_sb[:], in_=scal[:])
            swv_sb = cpool.tile([128, NSWV], f32)
            nc.sync.dma_start(out=swv_sb[:], in_=swv[:])
            swv_r = cpool.tile([128, NSWV], f32r)
            nc.vector.tensor_copy(out=swv_r[:], in_=swv_sb[:])
            ones_c = cpool.tile([128, 16], f32)
            nc.vector.memset(ones_c[:], 1.0)

            seq_sb = cpool.tile([128, NT], i32)
            nc.sync.dma_start(out=seq_sb[:], in_=seq_idx[:])
            cnd_sb = cpool.tile([128, NCT], i32)
            nc.sync.dma_start(out=cnd_sb[:], in_=cand_idx[:])

            ecw1_r = cpool.tile([128, 2, D], f32r)
            ecw2_r = cpool.tile([128, 2, ECO], f32r)
            for k in range(2):
                st = cpool.tile([128, D], f32, tag="wstage")
                nc.sync.dma_start(out=st[:], in_=ecw1[k])
                nc.vector.tensor_copy(out=ecw1_r[:, k, :], in_=st[:])
                st2 = cpool.tile([128, ECO], f32, tag="wstage2")
                nc.sync.dma_start(out=st2[:], in_=ecw2[k])
                nc.vector.tensor_copy(out=ecw2_r[:, k, :], in_=st2[:])
            sdw_r = cpool.tile([64, D], f32r)
            st = cpool.tile([64, D], f32, tag="wstage64")
            nc.sync.dma_start(out=st[:], in_=sdw[:])
            nc.vector.tensor_copy(out=sdw_r[:], in_=st[:])

            wt_dram = dram.tile([BL, 128, 4, S], f32)
            xfin_dram = dram.tile([T, D], f32)

            xpool_cm = tc.tile_pool(name="xpool", bufs=1)
            xpool = xpool_cm.__enter__()
            x_tok = xpool.tile([128, NT, D], f32)

            def fmbv(name, j=0):
                cix = _FMB_COLS[name] + j
                return fmb_sb[:, cix:cix + 1]

            def scv(j, p=128):
                return scal_sb[0:p, j:j + 1]

            # =========== Stage B: gather + embed ===========
            with tc.tile_pool(name="embp", bufs=3) as ep, \
                 tc.tile_pool(name="embps", bufs=2, space="PSUM") as eps_p:
                sideT = ep.tile([64, NT, 128], f32r, tag="sideT")
                for j in range(NT):
                    emb = ep.tile([128, D], f32, tag="emb")
                    nc.gpsimd.indirect_dma_start(
                        out=emb[:], out_offset=None, in_=item_tab[:],
                        in_offset=bass.IndirectOffsetOnAxis(
                            ap=seq_sb[:, j:j + 1], axis=0))
                    sg = ep.tile([128, SIDE], f32, tag="sg")
                    nc.gpsimd.indirect_dma_start(
                        out=sg[:], out_offset=None, in_=side_tab[:],
                        in_offset=bass.IndirectOffsetOnAxis(
                            ap=seq_sb[:, j:j + 1], axis=0))
                    nc.vector.tensor_tensor(out=x_tok[:, j, :], in0=emb[:],
                                            in1=pos_sb[:, j % 4, :], op=ALU.add)
                    tps = eps_p.tile([64, 128], f32, tag="tps")
                    nc.tensor.transpose(out=tps[:], in_=sg[:], identity=ident[:])
                    nc.vector.tensor_copy(out=sideT[:, j, :], in_=tps[:])
                for j in range(NT):
                    mmp = eps_p.tile([128, D], f32, tag="mmp")
                    nc.tensor.matmul(out=mmp[:], lhsT=sideT[:, j, :], rhs=sdw_r[:],
                                     start=True, stop=True)
                    nc.vector.tensor_tensor(out=x_tok[:, j, :], in0=x_tok[:, j, :],
                                            in1=mmp[:], op=ALU.add)
                    if "sideb" not in zflags:
                        nc.vector.tensor_tensor(out=x_tok[:, j, :],
                                                in0=x_tok[:, j, :],
                                                in1=bcc_sb[:, _BCC["sideb"], :],
                                                op=ALU.add)
            if dbg:
                nc.sync.dma_start(out=y_dx0[:], in_=x_tok[:, 0, :])

            # =========== Stage E: eco(x) -> gamma -> attention weights ===========
            gpool_cm = tc.tile_pool(name="gpool", bufs=1)
            gpool = gpool_cm.__enter__()
            gamma_row = gpool.tile([1, T], f32)
            with tc.tile_pool(name="ecop", bufs=2) as ecp, \
                 tc.tile_pool(name="ecops", bufs=1, space="PSUM") as ecps:
                for b in range(BL):
                    xTb = ecp.tile([128, 2, S], f32r, tag="xTb")
                    for tb in range(4):
                        for f in range(2):
                            tp = ecps.tile([128, 128], f32, tag="tp")
                            nc.tensor.transpose(
                                out=tp[:],
                                in_=x_tok[:, b * 4 + tb, f * 128:(f + 1) * 128],
                                identity=ident[:])
                            nc.scalar.copy(
                                out=xTb[:, f, tb * 128:(tb + 1) * 128], in_=tp[:])
                    h1 = ecp.tile([128, 2, S], f32r, tag="h1")
                    for f in range(2):
                        pm = ecps.tile([128, S], f32, tag="pm")
                        for k in range(2):
                            nc.tensor.matmul(
                                out=pm[:], lhsT=ecw1_r[:, k, f * 128:(f + 1) * 128],
                                rhs=xTb[:, k, :], start=(k == 0), stop=(k == 1))
                        nc.scalar.activation(out=h1[:, f, :], in_=pm[:], func=AF.Gelu,
                                             bias=fmbv("ecob1", f))
                    pm2 = ecps.tile([128, S], f32, tag="pm2")
                    for k in range(2):
                        nc.tensor.matmul(out=pm2[:], lhsT=ecw2_r[:, k, :],
                                         rhs=h1[:, k, :], start=(k == 0), stop=(k == 1))
                    ecoT = ecp.tile([128, S], f32r, tag="ecoT")
                    nc.vector.tensor_scalar(out=ecoT[:], in0=pm2[:],
                                            scalar1=fmbv("ecob2"), scalar2=None,
                                            op0=ALU.add)
                    pg = ecps.tile([1, S], f32, tag="pg")
                    nc.tensor.matmul(out=pg[:],
                                     lhsT=swv_r[:, _SWV["gamW"]:_SWV["gamW"] + 1],
                                     rhs=ecoT[:], start=True, stop=True)
                    nc.scalar.activation(out=gamma_row[0:1, b * S:(b + 1) * S],
                                         in_=pg[:], func=AF.Sigmoid,
                                         bias=scv(0, 1))
            if dbg:
                nc.sync.dma_start(out=y_dgam[:], in_=gamma_row[:])

            with tc.tile_pool(name="wtp", bufs=2) as wtp, \
                 tc.tile_pool(name="dtp", bufs=1) as dtp:
                deltaT = dtp.tile([128, 4, S], f32)
                for kt in range(4):
                    nc.sync.dma_start(out=deltaT[:, kt, :], in_=delta4[kt])
                for b in range(BL):
                    gbs = wtp.tile([128, S], f32, tag="gbs")
                    nc.gpsimd.partition_broadcast(
                        out_ap=gbs[:], in_ap=gamma_row[0:1, b * S:(b + 1) * S])
                    wtb = wtp.tile([128, 4, S], f32, tag="wtb")
                    nc.vector.tensor_tensor(
                        out=wtb[:, :, :],
                        in0=gbs[:].unsqueeze(1).to_broadcast([128, 4, S]),
                        in1=deltaT[:, :, :], op=ALU.mult)
                    nc.vector.tensor_scalar(out=wtb[:, :, :], in0=wtb[:, :, :],
                                            scalar1=scv(5), scalar2=None,
                                            op0=ALU.add)
                    nc.vector.reciprocal(out=wtb[:, :, :], in_=wtb[:, :, :])
                    nc.sync.dma_start(out=wt_dram[b], in_=wtb[:])
            gpool_cm.__exit__(None, None, None)

            # =========== Layers ===========
            def ln_phase(g_ix, b_ix, pool, spill=False):
                aggs = pool.tile([128, NT, 2], f32, tag="lnagg")
                for j in range(NT):
                    stt = pool.tile([128, 6], f32, tag="lnst")
                    nc.vector.bn_stats(out=stt[:], in_=x_tok[:, j, :])
                    nc.vector.bn_aggr(out=aggs[:, j, :], in_=stt[:])
                sd32 = pool.tile([128, NT, 1], f32, tag="lnsd")
                nc.scalar.activation(out=sd32[:], in_=aggs[:, :, 1:2],
                                     func=AF.Sqrt, bias=scv(3))
                rs32 = pool.tile([128, NT, 1], f32, tag="lnrs")
                nc.vector.reciprocal(out=rs32[:], in_=sd32[:])
                for j in range(NT):
                    nc.vector.tensor_scalar(out=x_tok[:, j, :],
                                            in0=x_tok[:, j, :],
                                            scalar1=aggs[:, j, 0:1],
                                            scalar2=rs32[:, j, 0:1],
                                            op0=ALU.subtract, op1=ALU.mult)
                    if "ln_affine" not in zflags:
                        nc.vector.tensor_tensor(out=x_tok[:, j, :],
                                                in0=x_tok[:, j, :],
                                                in1=bcc_sb[:, g_ix, :],
                                                op=ALU.mult)
                        nc.vector.tensor_tensor(out=x_tok[:, j, :],
                                                in0=x_tok[:, j, :],
                                                in1=bcc_sb[:, b_ix, :],
                                                op=ALU.add)
                    if spill:
                        nc.sync.dma_start(
                            out=xfin_dram[j * 128:(j + 1) * 128, :],
                            in_=x_tok[:, j, :])

            with tc.tile_pool(name="lw", bufs=1) as lw:
                for l in range(L):
                    qw_r = lw.tile([128, 2, D], f32r, tag="qw_r")
                    kw_r = lw.tile([128, 2, D], f32r, tag="kw_r")
                    vw_r = lw.tile([128, 2, D], f32r, tag="vw_r")
                    ow_r = lw.tile([128, 2, D], f32r, tag="ow_r")
                    f1_r = lw.tile([128, 2, 4 * D], f32r, tag="f1_r")
                    f2_r = lw.tile([128, 8, D], f32r, tag="f2_r")
                    for k in range(2):
                        for src, dst in [(qw, qw_r), (kw, kw_r), (vw, vw_r),
                                         (ow, ow_r)]:
                            stg = lw.tile([128, D], f32, tag="lstg")
                            nc.sync.dma_start(out=stg[:], in_=src[l, k])
                            nc.vector.tensor_copy(out=dst[:, k, :], in_=stg[:])
                        stg1 = lw.tile([128, 4 * D], f32, tag="lstg1")
                        nc.sync.dma_start(out=stg1[:], in_=f1w[l, k])
                        nc.vector.tensor_copy(out=f1_r[:, k, :], in_=stg1[:])
                    for k in range(8):
                        stg = lw.tile([128, D], f32, tag="lstg")
                        nc.sync.dma_start(out=stg[:], in_=f2w[l, k])
                        nc.vector.tensor_copy(out=f2_r[:, k, :], in_=stg[:])

                    # ---- phase A: attention (ACT: Exp) ----
                    with tc.tile_pool(name=f"at{l}", bufs=2) as ap, \
                         tc.tile_pool(name=f"a1_{l}", bufs=1) as a1, \
                         tc.tile_pool(name=f"atp{l}", bufs=1, space="PSUM") as aps:
                        for b in range(BL):
                            xTb = ap.tile([128, 2, S], f32r, tag="xTb")
                            for tb in range(4):
                                for f in range(2):
                                    tp = aps.tile([128, 128], f32, tag="tp")
                                    nc.tensor.transpose(
                                        out=tp[:],
                                        in_=x_tok[:, b * 4 + tb,
                                                  f * 128:(f + 1) * 128],
                                        identity=ident[:])
                                    nc.scalar.copy(
                                        out=xTb[:, f, tb * 128:(tb + 1) * 128],
                                        in_=tp[:])
                            qT = ap.tile([128, 2, S], f32r, tag="qT")
                            kT = ap.tile([128, 2, S], f32r, tag="kT")
                            for f in range(2):
                                pq = aps.tile([128, S], f32, tag="pq")
                                for k in range(2):
                                    nc.tensor.matmul(
                                        out=pq[:],
                                        lhsT=qw_r[:, k, f * 128:(f + 1) * 128],
                                        rhs=xTb[:, k, :], start=(k == 0),
                                        stop=(k == 1))
                                if "qb" in zflags:
                                    nc.scalar.copy(out=qT[:, f, :], in_=pq[:])
                                else:
                                    nc.vector.tensor_scalar(
                                        out=qT[:, f, :], in0=pq[:],
                                        scalar1=fmbv(f"qb{l}", f), scalar2=None,
                                        op0=ALU.add)
                                pk = aps.tile([128, S], f32, tag="pq")
                                for k in range(2):
                                    nc.tensor.matmul(
                                        out=pk[:],
                                        lhsT=kw_r[:, k, f * 128:(f + 1) * 128],
                                        rhs=xTb[:, k, :], start=(k == 0),
                                        stop=(k == 1))
                                if "kb" in zflags:
                                    nc.scalar.copy(out=kT[:, f, :], in_=pk[:])
                                else:
                                    nc.vector.tensor_scalar(
                                        out=kT[:, f, :], in0=pk[:],
                                        scalar1=fmbv(f"kb{l}", f), scalar2=None,
                                        op0=ALU.add)
                            vb_t = ap.tile([128, 4, 4 * 65], f32r, tag="vb_t")
                            for tb in range(4):
                                pv = aps.tile([128, D], f32, tag="pv")
                                for k in range(2):
                                    nc.tensor.matmul(
                                        out=pv[:],
                                        lhsT=xTb[:, k, tb * 128:(tb + 1) * 128],
                                        rhs=vw_r[:, k, :], start=(k == 0),
                                        stop=(k == 1))
                                dst = vb_t[:, tb, :].rearrange(
                                    "p (h c) -> p h c", h=4)[:, :, 0:64]
                                if "vb" in zflags:
                                    nc.vector.tensor_copy(
                                        out=dst,
                                        in_=pv[:].rearrange("p (h c) -> p h c",
                                                            h=4))
                                else:
                                    nc.vector.tensor_tensor(
                                        out=dst,
                                        in0=pv[:].rearrange("p (h c) -> p h c",
                                                            h=4),
                                        in1=bcc_sb[:, _BCC[f"vb{l}"], :].rearrange(
                                            "p (h c) -> p h c", h=4),
                                        op=ALU.add)
                                nc.vector.tensor_copy(
                                    out=vb_t[:, tb, :].rearrange(
                                        "p (h c) -> p h c", h=4)[:, :, 64:65],
                                    in_=ones_c[:, 0:4].unsqueeze(2))
                            wtb = ap.tile([128, 4, S], f32, tag="wtb")
                            nc.sync.dma_start(out=wtb[:], in_=wt_dram[b])
                            oTb = ap.tile([128, 2, S], f32r, tag="oTb")
                            for h in range(4):
                                hf, hp = h // 2, (h % 2) * 64
                                scp = aps.tile([128, 4, S], f32, tag="scp")
                                for kt in range(4):
                                    nc.tensor.matmul(
                                        out=scp[:, kt, :],
                                        lhsT=kT[hp:hp + 64, hf,
                                                kt * 128:(kt + 1) * 128],
                                        rhs=qT[hp:hp + 64, hf, :],
                                        start=True, stop=True)
                                eU = a1.tile([128, 4, S], f32, tag="eU")
                                nc.scalar.activation(out=eU[:], in_=scp[:],
                                                     func=AF.Exp)
                                aU = a1.tile([128, 4, S], f32r, tag="aU")
                                nc.vector.tensor_tensor(out=aU[:], in0=eU[:],
                                                        in1=wtb[:], op=ALU.mult)
                                avp = aps.tile([65, S], f32, tag="avp")
                                for kt in range(4):
                                    nc.tensor.matmul(
                                        out=avp[:],
                                        lhsT=vb_t[:, kt, h * 65:(h + 1) * 65],
                                        rhs=aU[:, kt, :], start=(kt == 0),
                                        stop=(kt == 3))
                                if dbg and l == 0 and b == 0 and h == 0:
                                    dat = a1.tile([128, S], f32, tag="dat")
                                    nc.vector.memset(dat[:], 0.0)
                                    nc.vector.tensor_copy(out=dat[0:65, :],
                                                          in_=avp[:])
                                    nc.sync.dma_start(out=y_dattn[:], in_=dat[:])
                                rr = a1.tile([1, S], f32, tag="rr")
                                nc.vector.reciprocal(out=rr[:], in_=avp[64:65, :])
                                rbs = a1.tile([64, S], f32, tag="rbs")
                                nc.gpsimd.partition_broadcast(out_ap=rbs[:],
                                                              in_ap=rr[:])
                                nc.vector.tensor_tensor(
                                    out=oTb[hp:hp + 64, hf, :], in0=avp[0:64, :],
                                    in1=rbs[:], op=ALU.mult)
                            for tb in range(4):
                                po = aps.tile([128, D], f32, tag="pv")
                                for f in range(2):
                                    nc.tensor.matmul(
                                        out=po[:],
                                        lhsT=oTb[:, f, tb * 128:(tb + 1) * 128],
                                        rhs=ow_r[:, f, :], start=(f == 0),
                                        stop=(f == 1))
                                j = b * 4 + tb
                                nc.vector.tensor_tensor(
                                    out=x_tok[:, j, :], in0=x_tok[:, j, :],
                                    in1=po[:], op=ALU.add)
                                if "ob" not in zflags:
                                    nc.vector.tensor_tensor(
                                        out=x_tok[:, j, :], in0=x_tok[:, j, :],
                                        in1=bcc_sb[:, _BCC[f"ob{l}"], :],
                                        op=ALU.add)

                    # ---- phase B: LN1 (ACT: Sqrt) ----
                    with tc.tile_pool(name=f"l1_{l}", bufs=2) as lnp:
                        ln_phase(_BCC[f"n1g{l}"], _BCC[f"n1b{l}"], lnp)

                    # ---- phase C: FF (ACT: Gelu) ----
                    with tc.tile_pool(name=f"ff{l}", bufs=2) as fp, \
                         tc.tile_pool(name=f"ffp{l}", bufs=1, space="PSUM") as fps:
                        for b in range(BL):
                            xTb = fp.tile([128, 2, S], f32r, tag="xTb")
                            for tb in range(4):
                                for f in range(2):
                                    tp = fps.tile([128, 128], f32, tag="tp")
                                    nc.tensor.transpose(
                                        out=tp[:],
                                        in_=x_tok[:, b * 4 + tb,
                                                  f * 128:(f + 1) * 128],
                                        identity=ident[:])
                                    nc.scalar.copy(
                                        out=xTb[:, f, tb * 128:(tb + 1) * 128],
                                        in_=tp[:])
                            hT = fp.tile([128, 8, S], f32r, tag="hT")
                            for fo in range(8):
                                pf = fps.tile([128, S], f32, tag="pf")
                                for k in range(2):
                                    nc.tensor.matmul(
                                        out=pf[:],
                                        lhsT=f1_r[:, k, fo * 128:(fo + 1) * 128],
                                        rhs=xTb[:, k, :], start=(k == 0),
                                        stop=(k == 1))
                                nc.scalar.activation(out=hT[:, fo, :], in_=pf[:],
                                                     func=AF.Gelu,
                                                     bias=fmbv(f"ffb1_{l}", fo))
                            for tb in range(4):
                                p2 = fps.tile([128, D], f32, tag="p2")
                                for k in range(8):
                                    nc.tensor.matmul(
                                        out=p2[:],
                                        lhsT=hT[:, k, tb * 128:(tb + 1) * 128],
                                        rhs=f2_r[:, k, :], start=(k == 0),
                                        stop=(k == 7))
                                j = b * 4 + tb
                                nc.vector.tensor_tensor(
                                    out=x_tok[:, j, :], in0=x_tok[:, j, :],
                                    in1=p2[:], op=ALU.add)
                                if "ffb2" not in zflags:
                                    nc.vector.tensor_tensor(
                                        out=x_tok[:, j, :], in0=x_tok[:, j, :],
                                        in1=bcc_sb[:, _BCC[f"ffb2_{l}"], :],
                                        op=ALU.add)

                    # ---- phase D: LN2 (ACT: Sqrt) ----
                    with tc.tile_pool(name=f"l2_{l}", bufs=2) as lnp:
                        ln_phase(_BCC[f"n2g{l}"], _BCC[f"n2b{l}"], lnp)
                    if dbg and l == 0:
                        nc.sync.dma_start(out=y_dl0[:], in_=x_tok[:, 0, :])

                # final LN + spill to DRAM
                with tc.tile_pool(name="lnf", bufs=2) as lnp:
                    ln_phase(_BCC["fng"], _BCC["fnb"], lnp, spill=True)
                if dbg:
                    nc.sync.dma_start(out=y_dxf[:], in_=x_tok[:, 0, :])
            xpool_cm.__exit__(None, None, None)

            # =========== Head ===========
            with tc.tile_pool(name="hd", bufs=1) as hd, \
                 tc.tile_pool(name="hdl", bufs=3) as hdl, \
                 tc.tile_pool(name="hps", bufs=2, space="PSUM") as hps, \
                 tc.tile_pool(name="hpl", bufs=1, space="PSUM") as hpl:

                def loadw(src, kdim, width, tg):
                    t = hd.tile([128, kdim, width], f32r, tag=tg)
                    for k in range(kdim):
                        stg = hdl.tile([128, width], f32, tag="hstg")
                        nc.sync.dma_start(out=stg[:], in_=src[k])
                        nc.vector.tensor_copy(out=t[:, k, :], in_=stg[:])
                    return t

                wi_r = loadw(wiw, 2, D, "wi_r")
                wi2_r = loadw(wi2w, 2, D, "wi2_r")
                smw1_r = loadw(smw1, 2, D, "smw1_r")
                smw2_r = loadw(smw2, 2, SEM, "smw2_r")
                wew_r = hd.tile([128, D], f32r, tag="wew_r")
                stg = hdl.tile([128, D], f32, tag="hstg")
                nc.sync.dma_start(out=stg[:], in_=wew[:])
                nc.vector.tensor_copy(out=wew_r[:], in_=stg[:])
                wsw_r = hd.tile([128, D], f32r, tag="wsw_r")
                stg = hdl.tile([128, D], f32, tag="hstg")
                nc.sync.dma_start(out=stg[:], in_=wsw[:])
                nc.vector.tensor_copy(out=wsw_r[:], in_=stg[:])
                pwp_sb = hd.tile([128, 2, P], f32)
                nc.sync.dma_start(out=pwp_sb[:], in_=pwp[:])
                prior_sb = hd.tile([8, CP, P], f32)
                nc.sync.dma_start(out=prior_sb[:], in_=prior_bc[:])

                li = hdl.tile([8, 1], i32, tag="li")
                nc.sync.dma_start(out=li[:], in_=last_idx[:])
                lastg = hdl.tile([8, D], f32, tag="lastg")
                nc.gpsimd.indirect_dma_start(
                    out=lastg[:], out_offset=None, in_=xfin_dram[:],
                    in_offset=bass.IndirectOffsetOnAxis(ap=li[:, :1], axis=0))
                lastT = hd.tile([128, 2, 8], f32r, tag="lastT")
                for f in range(2):
                    tp = hps.tile([128, 8], f32, tag="hmm")
                    nc.tensor.transpose(out=tp[:],
                                        in_=lastg[:, f * 128:(f + 1) * 128],
                                        identity=ident[0:8, 0:8])
                    nc.vector.tensor_copy(out=lastT[:, f, :], in_=tp[:])

                def mlp2(w1_r, b1n, w2_r, b2n, width2, tg):
                    h1 = hd.tile([128, 2, 8], f32r, tag=tg + "h")
                    for f in range(2):
                        pm = hps.tile([128, 8], f32, tag="hmm")
                        for k in range(2):
                            nc.tensor.matmul(
                                out=pm[:], lhsT=w1_r[:, k, f * 128:(f + 1) * 128],
                                rhs=lastT[:, k, :], start=(k == 0), stop=(k == 1))
                        nc.scalar.activation(out=h1[:, f, :], in_=pm[:],
                                             func=AF.Gelu, bias=fmbv(b1n, f))
                    pm2 = hps.tile([128, 8], f32, tag="hmm")
                    for k in range(2):
                        nc.tensor.matmul(out=pm2[:], lhsT=w2_r[:, k, 0:width2],
                                         rhs=h1[:, k, :], start=(k == 0),
                                         stop=(k == 1))
                    out = hd.tile([128, 8], f32r, tag=tg)
                    nc.vector.tensor_scalar(out=out[0:width2, :],
                                            in0=pm2[0:width2, :],
                                            scalar1=fmbv(b2n)[0:width2, :],
                                            scalar2=None, op0=ALU.add)
                    return out

                ecoS = mlp2(ecw1_r, "ecob1", ecw2_r, "ecob2", ECO, "ecoS")
                semS = mlp2(smw1_r, "semb1", smw2_r, "semb2", SEM, "semS")
                if dbg:
                    dcs = hdl.tile([128, 8], f32, tag="dcs")
                    nc.vector.tensor_copy(out=dcs[:], in_=ecoS[:])
                    nc.sync.dma_start(out=y_decs[:], in_=dcs[:])

                prp = hd.tile([128, NCT], f32, tag="prp")
                icT = hd.tile([128, 2, TCD], f32r, tag="icT")
                for j in range(NCT):
                    ce = hdl.tile([128, D], f32, tag="ce")
                    nc.gpsimd.indirect_dma_start(
                        out=ce[:], out_offset=None, in_=item_tab[:],
                        in_offset=bass.IndirectOffsetOnAxis(
                            ap=cnd_sb[:, j:j + 1], axis=0))
                    cs = hdl.tile([128, SIDE], f32, tag="cs")
                    nc.gpsimd.indirect_dma_start(
                        out=cs[:], out_offset=None, in_=side_tab[:],
                        in_offset=bass.IndirectOffsetOnAxis(
                            ap=cnd_sb[:, j:j + 1], axis=0))
                    pp = hdl.tile([128, 1], f32, tag="pp")
                    nc.gpsimd.indirect_dma_start(
                        out=pp[:], out_offset=None, in_=prop_tab[:],
                        in_offset=bass.IndirectOffsetOnAxis(
                            ap=cnd_sb[:, j:j + 1], axis=0))
                    nc.vector.tensor_copy(out=prp[:, j:j + 1], in_=pp[:])
                    tps = hps.tile([64, 128], f32, tag="hmm")
                    nc.tensor.transpose(out=tps[:], in_=cs[:], identity=ident[:])
                    csT = hdl.tile([64, 128], f32r, tag="csT")
                    nc.vector.tensor_copy(out=csT[:], in_=tps[:])
                    pm = hps.tile([128, D], f32, tag="hmm")
                    nc.tensor.matmul(out=pm[:], lhsT=csT[:], rhs=sdw_r[:],
                                     start=True, stop=True)
                    ic = hdl.tile([128, D], f32, tag="ic")
                    nc.vector.tensor_tensor(out=ic[:], in0=ce[:],
                                            in1=pm[:], op=ALU.add)
                    if "sideb" not in zflags:
                        nc.vector.tensor_tensor(out=ic[:], in0=ic[:],
                                                in1=bcc_sb[:, _BCC["sideb"], :],
                                                op=ALU.add)
                    for f in range(2):
                        tp = hps.tile([128, 128], f32, tag="hmm")
                        nc.tensor.transpose(out=tp[:],
                                            in_=ic[:, f * 128:(f + 1) * 128],
                                            identity=ident[:])
                        nc.vector.tensor_copy(
                            out=icT[:, f, j * 128:(j + 1) * 128], in_=tp[:])

                icWT = hd.tile([128, 2, TCD], f32, tag="icWT")
                eWeT = hd.tile([128, 2, 8], f32, tag="eWeT")
                swbT = hd.tile([128, 2, 8], f32, tag="swbT")
                for f in range(2):
                    for ch in range(4):
                        pm = hps.tile([128, S], f32, tag="hmm")
                        for k in range(2):
                            nc.tensor.matmul(
                                out=pm[:], lhsT=wi_r[:, k, f * 128:(f + 1) * 128],
                                rhs=icT[:, k, ch * S:(ch + 1) * S],
                                start=(k == 0), stop=(k == 1))
                        nc.vector.tensor_scalar(
                            out=icWT[:, f, ch * S:(ch + 1) * S], in0=pm[:],
                            scalar1=fmbv("dob", f), scalar2=None, op0=ALU.add)
                    pe = hps.tile([128, 8], f32, tag="hmm")
                    nc.tensor.matmul(out=pe[:],
                                     lhsT=wew_r[:, f * 128:(f + 1) * 128],
                                     rhs=ecoS[0:ECO, :], start=True, stop=True)
                    nc.vector.tensor_copy(out=eWeT[:, f, :], in_=pe[:])
                    ps_ = hps.tile([128, 8], f32, tag="hmm")
                    nc.tensor.matmul(out=ps_[:],
                                     lhsT=wsw_r[:, f * 128:(f + 1) * 128],
                                     rhs=semS[0:SEM, :], start=True, stop=True)
                    nc.vector.tensor_scalar(out=swbT[:, f, :], in0=ps_[:],
                                            scalar1=fmbv("shb1", f), scalar2=None,
                                            op0=ALU.add)

                aT = hd.tile([128, 2, 8 * P], f32, tag="aT")
                for f in range(2):
                    nc.vector.tensor_tensor(
                        out=aT[:, f, :].rearrange("d (b p) -> d b p", b=8),
                        in0=eWeT[:, f, :].unsqueeze(2).to_broadcast([128, 8, P]),
                        in1=pwp_sb[:, f, :].unsqueeze(1).to_broadcast([128, 8, P]),
                        op=ALU.add)

                hsh = hd.tile([128, 2, TCD], f32r, tag="hsh")
                for f in range(2):
                    for b in range(BL):
                        pm2 = hps.tile([128, CP], f32, tag="hmm")
                        for k in range(2):
                            nc.tensor.matmul(
                                out=pm2[:], lhsT=wi2_r[:, k, f * 128:(f + 1) * 128],
                                rhs=icT[:, k, b * CP:(b + 1) * CP],
                                start=(k == 0), stop=(k == 1))
                        nc.scalar.activation(out=hsh[:, f, b * CP:(b + 1) * CP],
                                             in_=pm2[:],
                                             func=AF.Gelu, bias=swbT[:, f, b:b + 1])

                lg8 = hd.tile([8, CP, P], f32, tag="lg8")
                sh8 = hd.tile([8, CP], f32, tag="sh8")
                for b in range(BL):
                    preT = hd.tile([128, 2, CP * P], f32, tag="preT")
                    for f in range(2):
                        nc.vector.tensor_tensor(
                            out=preT[:, f, :].rearrange("d (c p) -> d c p", c=CP),
                            in0=icWT[:, f, b * CP:(b + 1) * CP].unsqueeze(2)
                                .to_broadcast([128, CP, P]),
                            in1=aT[:, f, b * P:(b + 1) * P].unsqueeze(1)
                                .to_broadcast([128, CP, P]),
                            op=ALU.add)
                    tnh = hd.tile([128, 2, CP * P], f32r, tag="tnh")
                    nc.scalar.activation(out=tnh[:], in_=preT[:], func=AF.Tanh)
                    lgp = hpl.tile([1, CP * P], f32, tag="lgp")
                    for ch in range(4):
                        for f in range(2):
                            nc.tensor.matmul(
                                out=lgp[0:1, ch * S:(ch + 1) * S],
                                lhsT=swv_r[:, _SWV[f"doOutW_{f}"]:
                                           _SWV[f"doOutW_{f}"] + 1],
                                rhs=tnh[:, f, ch * S:(ch + 1) * S],
                                start=(f == 0), stop=(f == 1))
                    lrow = hd.tile([1, CP * P], f32, tag="lrow")
                    nc.vector.tensor_copy(out=lrow[:], in_=lgp[:])
                    nc.sync.dma_start(
                        out=lg8[b:b + 1, :, :],
                        in_=lrow[:].rearrange("a (c p) -> a c p", c=CP))
                    shp = hps.tile([1, CP], f32, tag="hmm")
                    for f in range(2):
                        nc.tensor.matmul(
                            out=shp[:],
                            lhsT=swv_r[:, _SWV[f"shW2_{f}"]:_SWV[f"shW2_{f}"] + 1],
                            rhs=hsh[:, f, b * CP:(b + 1) * CP],
                            start=(f == 0), stop=(f == 1))
                    srow = hdl.tile([1, CP], f32, tag="srow")
                    nc.vector.tensor_scalar(out=srow[:], in0=shp[:],
                                            scalar1=scv(7, 1), scalar2=None,
                                            op0=ALU.add)
                    nc.sync.dma_start(out=sh8[b:b + 1, :], in_=srow[:])

                do8 = hd.tile([8, CP], f32, tag="do8")
                lgw = hd.tile([8, CP, P], f32, tag="lgw")
                nc.vector.tensor_tensor(out=lgw[:], in0=lg8[:], in1=prior_sb[:],
                                        op=ALU.mult)
                nc.vector.tensor_reduce(out=do8[:], in_=lgw[:], axis=AX.X,
                                        op=ALU.add)
                nc.vector.tensor_scalar(out=do8[:], in0=do8[:], scalar1=scv(6, 8),
                                        scalar2=None, op0=ALU.add)

                mabs = hdl.tile([8, 1], f32, tag="mabs")
                nc.vector.tensor_reduce(out=mabs[:], in_=do8[:, 0:C], axis=AX.X,
                                        op=ALU.add)
                nc.vector.tensor_scalar(out=mabs[:], in0=mabs[:], scalar1=1.0 / C,
                                        scalar2=None, op0=ALU.mult)

                br_p = hps.tile([1, 8], f32, tag="hmm")
                nc.tensor.matmul(out=br_p[:],
                                 lhsT=swv_r[:, _SWV["refW"]:_SWV["refW"] + 1],
                                 rhs=ecoS[0:ECO, :], start=True, stop=True)
                bref_row = hdl.tile([1, 8], f32, tag="bref_row")
                nc.vector.tensor_scalar(out=bref_row[:], in0=br_p[:],
                                        scalar1=scv(1, 1), scalar2=None,
                                        op0=ALU.add)
                et_p = hps.tile([1, 8], f32, tag="hmm")
                nc.tensor.matmul(out=et_p[:],
                                 lhsT=swv_r[:, _SWV["etaW"]:_SWV["etaW"] + 1],
                                 rhs=ecoS[0:ECO, :], start=True, stop=True)
                # mabs -> row layout
                mb_p = hps.tile([1, 8], f32, tag="hmm")
                nc.tensor.transpose(out=mb_p[:], in_=mabs[:],
                                    identity=ident[0:8, 0:8])
                mabs_row = hdl.tile([1, 8], f32, tag="mabs_row")
                nc.vector.tensor_copy(out=mabs_row[:], in_=mb_p[:])
                # eta = sigmoid(etaraw + etaW_last*mabs + etab)  (row layout)
                etav = hdl.tile([1, 8], f32, tag="etav")
                nc.vector.tensor_scalar(out=etav[:], in0=mabs_row[:],
                                        scalar1=scv(8, 1), scalar2=None,
                                        op0=ALU.mult)
                nc.vector.tensor_tensor(out=etav[:], in0=etav[:],
                                        in1=et_p[:], op=ALU.add)
                eta = hdl.tile([1, 8], f32, tag="eta")
                nc.scalar.activation(out=eta[:], in_=etav[:], func=AF.Sigmoid,
                                     bias=scv(2, 1))
                # ref = bref + eta*(mabs - bref)   (row layout)
                dref = hdl.tile([1, 8], f32, tag="dref")
                nc.vector.tensor_tensor(out=dref[:], in0=mabs_row[:],
                                        in1=bref_row[:], op=ALU.subtract)
                nc.vector.tensor_tensor(out=dref[:], in0=dref[:], in1=eta[:],
                                        op=ALU.mult)
                ref_row = hdl.tile([1, 8], f32, tag="ref_row")
                nc.vector.tensor_tensor(out=ref_row[:], in0=bref_row[:],
                                        in1=dref[:], op=ALU.add)
                rf_p = hps.tile([8, 1], f32, tag="hmm")
                nc.tensor.transpose(out=rf_p[:], in_=ref_row[:],
                                    identity=ident[0:1, 0:1])
                ref8 = hdl.tile([8, 1], f32, tag="ref8")
                nc.vector.tensor_copy(out=ref8[:], in_=rf_p[:])
                nc.sync.dma_start(out=y_ref[:], in_=ref8[:, 0:1])

                # prospect value
                du = hd.tile([8, CP], f32, tag="du")
                nc.vector.tensor_scalar(out=du[:], in0=do8[:], scalar1=ref8[:, 0:1],
                                        scalar2=None, op0=ALU.subtract)
                gate = hd.tile([8, CP], f32, tag="gate")
                nc.scalar.activation(out=gate[:], in_=du[:], func=AF.Sigmoid,
                                     scale=1.0 / GTEMP)
                du2l = hd.tile([8, CP], f32, tag="du2l")
                nc.scalar.activation(out=du2l[:], in_=du[:], func=AF.Square)
                nc.scalar.activation(out=du2l[:], in_=du2l[:], func=AF.Ln,
                                     bias=scv(4, 8))
                gain = hd.tile([8, CP], f32, tag="gain")
                nc.scalar.activation(out=gain[:], in_=du2l[:], func=AF.Exp,
                                     scale=ALPHA / 2.0)
                lossv = hd.tile([8, CP], f32, tag="lossv")
                nc.scalar.activation(out=lossv[:], in_=du2l[:], func=AF.Exp,
                                     scale=BETA / 2.0)
                longs = hd.tile([8, CP], f32, tag="longs")
                t2 = hd.tile([8, CP], f32, tag="t2")
                nc.vector.tensor_tensor(out=longs[:], in0=gate[:], in1=gain[:],
                                        op=ALU.mult)
                nc.vector.tensor_tensor(out=t2[:], in0=gate[:], in1=lossv[:],
                                        op=ALU.mult)
                nc.vector.tensor_tensor(out=t2[:], in0=t2[:], in1=lossv[:],
                                        op=ALU.subtract)
                nc.vector.tensor_scalar(out=t2[:], in0=t2[:], scalar1=LAM,
                                        scalar2=None, op0=ALU.mult)
                nc.vector.tensor_tensor(out=longs[:], in0=longs[:], in1=t2[:],
                                        op=ALU.add)
                tot8 = hd.tile([8, CP], f32, tag="tot8")
                nc.vector.tensor_scalar(out=tot8[:], in0=longs[:],
                                        scalar1=scv(9, 8), scalar2=None,
                                        op0=ALU.mult)
                nc.vector.tensor_tensor(out=tot8[:], in0=tot8[:], in1=sh8[:],
                                        op=ALU.add)

                # w: transpose prop [128,16] -> [8,256]
                prT_p = hps.tile([16, 128], f32, tag="hmm")
                nc.tensor.transpose(out=prT_p[:], in_=prp[:], identity=ident[:])
                prT = hdl.tile([16, 128], f32, tag="prT")
                nc.vector.tensor_copy(out=prT[:], in_=prT_p[:])
                prp8 = hd.tile([8, 2, 128], f32, tag="prp8")
                for b in range(8):
                    for hi in range(2):
                        nc.sync.dma_start(
                            out=prp8[b:b + 1, hi, :],
                            in_=prT[2 * b + hi:2 * b + hi + 1, :])
                wv = hd.tile([8, CP], f32, tag="wv")
                nc.vector.tensor_scalar(out=wv[:], in0=prp8[:].rearrange(
                    "b h c -> b (h c)"), scalar1=1e-6, scalar2=None, op0=ALU.max)
                nc.vector.reciprocal(out=wv[:], in_=wv[:])
                nc.vector.tensor_scalar(out=wv[:], in0=wv[:], scalar1=TAU,
                                        scalar2=None, op0=ALU.min)
                wm = hdl.tile([8, 1], f32, tag="wm")
                nc.vector.tensor_reduce(out=wm[:], in_=wv[:, 0:C], axis=AX.X,
                                        op=ALU.add)
                nc.vector.tensor_scalar(out=wm[:], in0=wm[:], scalar1=1.0 / C,
                                        scalar2=1e-8, op0=ALU.mult, op1=ALU.add)
                nc.vector.reciprocal(out=wm[:], in_=wm[:])
                w8 = hd.tile([8, CP], f32, tag="w8")
                nc.vector.tensor_scalar(out=w8[:], in0=wv[:], scalar1=wm[:, 0:1],
                                        scalar2=None, op0=ALU.mult)

                # ortho partials
                prod = hd.tile([128, 8], f32r, tag="prod")
                nc.vector.tensor_tensor(out=prod[:], in0=semS[:], in1=ecoS[:],
                                        op=ALU.mult)
                os_p = hps.tile([1, 8], f32, tag="hmm")
                nc.tensor.matmul(out=os_p[:],
                                 lhsT=swv_r[:, _SWV["ones"]:_SWV["ones"] + 1],
                                 rhs=prod[:], start=True, stop=True)
                osq = hdl.tile([1, 8], f32, tag="osq")
                nc.scalar.activation(out=osq[:], in_=os_p[:], func=AF.Square)
                nc.sync.dma_start(out=y_osq[:], in_=osq[0:1, :])

                nc.sync.dma_start(out=y_total[:], in_=tot8[:, 0:C])
                nc.sync.dma_start(out=y_short[:], in_=sh8[:, 0:C])
                nc.sync.dma_start(out=y_long[:], in_=longs[:, 0:C])
                nc.sync.dma_start(out=y_do[:], in_=do8[:, 0:C])
                nc.sync.dma_start(out=y_w[:], in_=w8[:, 0:C])

    nc.finalize()
    return nc


def _np_softmax(x):
    e = np.exp(x - x.max())
    return e / e.sum()


def _prep_host(inputs):
    """Shared (core-independent) host-side tensor prep."""
    g = {k: np.asarray(v) for k, v in inputs.items()}
    item_tab = np.ascontiguousarray(g["item_table"].astype(np.float32))
    item_tab = item_tab.copy()
    item_tab[0, :] = 0.0
    side_tab = np.ascontiguousarray(g["side_feat"].astype(np.float32))
    prop_tab = np.ascontiguousarray(
        g["prop"].astype(np.float32).reshape(NI + 1, 1))

    pos4 = np.ascontiguousarray(
        g["pos_table"].astype(np.float32).reshape(4, 128, D))

    kk = np.arange(S)[:, None]
    qq = np.arange(S)[None, :]
    dm = np.where(kk <= qq, (qq - kk).astype(np.float32), np.float32(1e35))
    delta4 = np.ascontiguousarray(dm.reshape(4, 128, S).astype(np.float32))

    sc = 1.0 / np.sqrt(DH)
    qw = np.ascontiguousarray(
        (g["qW"] * sc).astype(np.float32).reshape(L, 2, 128, D))
    kw = np.ascontiguousarray(g["kW"].astype(np.float32).reshape(L, 2, 128, D))
    vw = np.ascontiguousarray(g["vW"].astype(np.float32).reshape(L, 2, 128, D))
    ow = np.ascontiguousarray(g["oW"].astype(np.float32).reshape(L, 2, 128, D))
    f1w = np.ascontiguousarray(
        g["ffW1"].astype(np.float32).reshape(L, 2, 128, 4 * D))
    f2w = np.ascontiguousarray(
        g["ffW2"].astype(np.float32).reshape(L, 8, 128, D))
    ecw1 = np.ascontiguousarray(g["ecoW1"].astype(np.float32).reshape(2, 128, D))
    ecw2 = np.ascontiguousarray(
        g["ecoW2"].astype(np.float32).reshape(2, 128, ECO))
    smw1 = np.ascontiguousarray(g["semW1"].astype(np.float32).reshape(2, 128, D))
    smw2 = np.ascontiguousarray(
        g["semW2"].astype(np.float32).reshape(2, 128, SEM))
    sdw = np.ascontiguousarray(g["side_W"].astype(np.float32))
    doW = g["doW"].astype(np.float32)
    wiw = np.ascontiguousarray(doW[:D].reshape(2, 128, D))
    wew = np.ascontiguousarray(doW[D:D + ECO])
    shW1 = g["shW1"].astype(np.float32)
    wsw = np.ascontiguousarray(shW1[:SEM])
    wi2w = np.ascontiguousarray(shW1[SEM:].reshape(2, 128, D))

    swv = np.zeros((128, NSWV), np.float32)
    swv[:, _SWV["gamW"]] = g["gamW"][:, 0]
    swv[:, _SWV["refW"]] = g["refW"][:, 0]
    swv[:, _SWV["etaW"]] = g["etaW"][:ECO, 0]
    shW2 = g["shW2"].astype(np.float32)
    swv[:, _SWV["shW2_0"]] = shW2[:128, 0]
    swv[:, _SWV["shW2_1"]] = shW2[128:, 0]
    doOutW = g["doOutW"].astype(np.float32)
    swv[:, _SWV["doOutW_0"]] = doOutW[:128, 0]
    swv[:, _SWV["doOutW_1"]] = doOutW[128:, 0]
    swv[:, _SWV["ones"]] = 1.0

    fmb = np.zeros((128, NFMB), np.float32)

    def setf(nm, vec):
        n = len(vec) // 128
        for j in range(n):
            fmb[:, _FMB_COLS[nm] + j] = vec[j * 128:(j + 1) * 128]

    setf("qb0", g["qb"][0] * sc); setf("qb1", g["qb"][1] * sc)
    setf("kb0", g["kb"][0]); setf("kb1", g["kb"][1])
    setf("ecob1", g["ecob1"]); setf("ecob2", g["ecob2"])
    setf("semb1", g["semb1"]); setf("semb2", g["semb2"])
    setf("ffb1_0", g["ffb1"][0]); setf("ffb1_1", g["ffb1"][1])
    setf("dob", g["dob"]); setf("shb1", g["shb1"]); setf("sideb", g["side_b"])

    bcc = np.zeros((NBCC, 128, D), np.float32)

    def setb(nm, vec):
        bcc[_BCC[nm]] = np.tile(np.asarray(vec, np.float32)[None, :], (128, 1))

    setb("sideb", g["side_b"])
    for l in range(L):
        setb(f"vb{l}", g["vb"][l]); setb(f"ob{l}", g["ob"][l])
        setb(f"ffb2_{l}", g["ffb2"][l])
        setb(f"n1g{l}", g["n1g"][l]); setb(f"n1b{l}", g["n1b"][l])
        setb(f"n2g{l}", g["n2g"][l]); setb(f"n2b{l}", g["n2b"][l])
    setb("fng", g["fng"]); setb("fnb", g["fnb"])

    prior = _np_softmax(np.asarray(g["plog"], np.float32))
    prior_bc = np.ascontiguousarray(
        np.broadcast_to(prior[None, None, :], (8, CP, P)).astype(np.float32))
    pwpT = (np.asarray(g["proto"], np.float32) @ doW[D + ECO:]).T  # [D, P]
    pwp = np.ascontiguousarray(pwpT.reshape(2, 128, P).transpose(1, 0, 2))

    kappa = float(np.asarray(g["kappa"]))
    kscale = 1.0 / (1.0 + np.log1p(np.exp(kappa)))
    scal = np.zeros((128, NSCAL), np.float32)
    row = [float(g["gamb"][0]), float(g["refb"][0]), float(g["etab"][0]),
           1e-5, 1e-6, 1.0 + 1e-8, float(g["doOutb"][0]), float(g["shb2"][0]),
           float(g["etaW"][ECO, 0]), kscale]
    scal[:] = np.asarray(row, np.float32)[None, :]

    zf = set()
    if (np.all(g["n1g"] == 1) and np.all(g["n2g"] == 1) and np.all(g["fng"] == 1)
            and np.all(g["n1b"] == 0) and np.all(g["n2b"] == 0)
            and np.all(g["fnb"] == 0)):
        zf.add("ln_affine")
    if np.all(g["side_b"] == 0):
        zf.add("sideb")
    if np.all(g["qb"] == 0):
        zf.add("qb")
    if np.all(g["kb"] == 0):
        zf.add("kb")
    if np.all(g["vb"] == 0):
        zf.add("vb")
    if np.all(g["ob"] == 0):
        zf.add("ob")
    if np.all(g["ffb2"] == 0):
        zf.add("ffb2")
    zflags = frozenset(zf)

    shared = dict(item_tab=item_tab, side_tab=side_tab, prop_tab=prop_tab,
                  pos4=pos4, delta4=delta4, qw=qw, kw=kw, vw=vw, ow=ow,
                  f1w=f1w, f2w=f2w, ecw1=ecw1, ecw2=ecw2, smw1=smw1,
                  smw2=smw2, sdw=sdw, wiw=wiw, wew=wew, wi2w=wi2w, wsw=wsw,
                  swv=swv, fmb=fmb, bcc=bcc, prior_bc=prior_bc, pwp=pwp,
                  scal=scal)

    seq = np.asarray(g["seq"]).astype(np.int64)
    cand = np.asarray(g["candidate_items"]).astype(np.int64)
    lens = np.maximum((seq != 0).sum(1), 1)
    in_maps = []
    for c in range(NCORE):
        sl = slice(c * BL, (c + 1) * BL)
        seq_c = seq[sl].reshape(T).astype(np.int32)
        seq_ix = np.ascontiguousarray(seq_c.reshape(NT, 128).T)
        cand_pad = np.zeros((BL, CP), np.int32)
        cand_pad[:, :C] = cand[sl]
        cand_ix = np.ascontiguousarray(
            cand_pad.reshape(TCD).reshape(NCT, 128).T)
        last_ix = (np.arange(BL) * S + lens[sl] - 1).astype(
            np.int32).reshape(8, 1)
        m = dict(shared)
        m["seq_idx"] = seq_ix
        m["cand_idx"] = cand_ix
        m["last_idx"] = np.ascontiguousarray(last_ix)
        in_maps.append(m)
    return in_maps, zflags


def kernel(dbg=False, _res_out=None, **inputs):
    in_maps, zflags = _prep_host(inputs)
    key = (bool(dbg), zflags)
    if key not in _PROG_CACHE:
        _PROG_CACHE[key] = _build_program(dbg=key[0], zflags=zflags)
    nc = _PROG_CACHE[key]
    res = run_bass_kernel_spmd(nc, in_maps, list(range(NCORE)))
    if _res_out is not None:
        _res_out.append(res)
    rs = res.results
    total = np.concatenate([rs[c]["y_total"] for c in range(NCORE)], 0)
    short_s = np.concatenate([rs[c]["y_short"] for c in range(NCORE)], 0)
    long_s = np.concatenate([rs[c]["y_long"] for c in range(NCORE)], 0)
    do_util = np.concatenate([rs[c]["y_do"] for c in range(NCORE)], 0)
    ref = np.concatenate([rs[c]["y_ref"] for c in range(NCORE)], 0)
    w = np.concatenate([rs[c]["y_w"] for c in range(NCORE)], 0)
    osq = np.concatenate([rs[c]["y_osq"] for c in range(NCORE)], 0)
    ortho = np.float32(osq.mean())
    return (total.astype(np.float32), short_s.astype(np.float32),
            long_s.astype(np.float32), do_util.astype(np.float32),
            ref.astype(np.float32), w.astype(np.float32), ortho)


# revision 12
# speedup vs baseline: 1.1314x; 1.1314x over previous
"""Trainium2 Bass kernel for nn_CEINN_67138928771458.

Data-parallel over batch across 8 NeuronCores (8 batches/core), params
replicated. Activations kept token-major ([128 tokens, feat] tiles); PE
transposes produce feature-major matmul operands. Matmuls run in float32r
(full-rate fp32 with mantissa rounding). Attention uses the identity
softmax(s + hyper) = exp(s)*exp(hyper) / sum, with exp(hyper) =
1/(1 + gamma*delta + 1e-8) precomputed per batch (masked entries get
delta=1e35 so the weight underflows to ~0).
"""
import sys

sys.path.insert(0, "/opt/trn_rl_repo")

import numpy as np
import concourse.bass as bass
import concourse.tile as tile
from concourse import mybir, bacc
from concourse.bass_utils import run_bass_kernel_spmd
from concourse.masks import make_identity

f32 = mybir.dt.float32
f32r = mybir.dt.float32r
i32 = mybir.dt.int32
AF = mybir.ActivationFunctionType
ALU = mybir.AluOpType
AX = mybir.AxisListType

B, S, C, D, H, L = 64, 512, 200, 256, 4, 2
SEM, ECO, P, NI, SIDE, DH = 128, 128, 8, 50000, 64, 64
ALPHA, BETA, LAM, GTEMP, TAU = 0.7, 0.8, 2.0, 0.5, 10.0
NCORE, BL = 8, 8
T = BL * S
NT = T // 128
CP = 256
TCD = BL * CP
NCT = TCD // 128

_FMB_COLS = {}
_c = 0
for _nm, _n in [("qb0", 2), ("qb1", 2), ("kb0", 2), ("kb1", 2),
                ("ecob1", 2), ("ecob2", 1), ("semb1", 2), ("semb2", 1),
                ("ffb1_0", 8), ("ffb1_1", 8), ("dob", 2), ("shb1", 2),
                ("sideb", 2)]:
    _FMB_COLS[_nm] = _c
    _c += _n
NFMB = _c

_BCC = {}
for _i, _nm in enumerate(["sideb", "vb0", "vb1", "ob0", "ob1", "ffb2_0",
                          "ffb2_1", "n1g0", "n1b0", "n2g0", "n2b0",
                          "n1g1", "n1b1", "n2g1", "n2b1", "fng", "fnb"]):
    _BCC[_nm] = _i
NBCC = len(_BCC)

_SWV = {"gamW": 0, "refW": 1, "etaW": 2, "shW2_0": 3, "shW2_1": 4,
        "doOutW_0": 5, "doOutW_1": 6, "ones": 7}
NSWV = 8
# scal cols: 0=gamb 1=refb 2=etab 3=1e-5 4=1e-6 5=1+1e-8 6=doOutb 7=shb2
#            8=etaW_last 9=kscale
NSCAL = 10

_PROG_CACHE = {}


def _build_program(dbg=False, zflags=frozenset()):
    nc = bacc.Bacc(None, target_bir_lowering=False)

    def din(name, shape, dtype=f32):
        return nc.declare_dram_parameter(name, list(shape), dtype, isOutput=False)

    def dout(name, shape, dtype=f32):
        return nc.declare_dram_parameter(name, list(shape), dtype, isOutput=True)

    item_tab = din("item_tab", [NI + 1, D])
    side_tab = din("side_tab", [NI + 1, SIDE])
    prop_tab = din("prop_tab", [NI + 1, 1])
    seq_idx = din("seq_idx", [128, NT], i32)
    cand_idx = din("cand_idx", [128, NCT], i32)
    last_idx = din("last_idx", [8, 1], i32)
    pos4 = din("pos4", [4, 128, D])
    delta4 = din("delta4", [4, 128, S])
    qw = din("qw", [L, 2, 128, D]); kw = din("kw", [L, 2, 128, D])
    vw = din("vw", [L, 2, 128, D]); ow = din("ow", [L, 2, 128, D])
    f1w = din("f1w", [L, 2, 128, 4 * D]); f2w = din("f2w", [L, 8, 128, D])
    ecw1 = din("ecw1", [2, 128, D]); ecw2 = din("ecw2", [2, 128, ECO])
    smw1 = din("smw1", [2, 128, D]); smw2 = din("smw2", [2, 128, SEM])
    sdw = din("sdw", [SIDE, D])
    wiw = din("wiw", [2, 128, D]); wew = din("wew", [128, D])
    wi2w = din("wi2w", [2, 128, D]); wsw = din("wsw", [128, D])
    swv = din("swv", [128, NSWV])
    fmb = din("fmb", [128, NFMB])
    bcc = din("bcc", [NBCC, 128, D])
    prior_bc = din("prior_bc", [8, CP, P])
    pwp = din("pwp", [128, 2, P])
    scal = din("scal", [128, NSCAL])

    y_total = dout("y_total", [BL, C]); y_short = dout("y_short", [BL, C])
    y_long = dout("y_long", [BL, C]); y_do = dout("y_do", [BL, C])
    y_ref = dout("y_ref", [BL]); y_w = dout("y_w", [BL, C])
    y_osq = dout("y_osq", [BL])
    if dbg:
        y_dx0 = dout("y_dx0", [128, D])
        y_dgam = dout("y_dgam", [1, T])
        y_dl0 = dout("y_dl0", [128, D])
        y_dxf = dout("y_dxf", [128, D])
        y_decs = dout("y_decs", [128, 8])
        y_dattn = dout("y_dattn", [128, S])

    with tile.TileContext(nc) as tc:
        with tc.tile_pool(name="cpool", bufs=1) as cpool, \
             tc.tile_pool(name="dram", bufs=1, space="DRAM") as dram:

            ident = cpool.tile([128, 128], f32)
            make_identity(nc, ident)

            pos_sb = cpool.tile([128, 4, D], f32)
            for j in range(4):
                nc.sync.dma_start(out=pos_sb[:, j, :], in_=pos4[j])
            bcc_sb = cpool.tile([128, NBCC, D], f32)
            for j in range(NBCC):
                nc.sync.dma_start(out=bcc_sb[:, j, :], in_=bcc[j])
            fmb_sb = cpool.tile([128, NFMB], f32)
            nc.sync.dma_start(out=fmb_sb[:], in_=fmb[:])
            scal_sb = cpool.tile([128, NSCAL], f32)
            nc.sync.dma_start(out=scal_sb[:], in_=scal[:])
            swv_sb = cpool.tile([128, NSWV], f32)
            nc.sync.dma_start(out=swv_sb[:], in_=swv[:])
            swv_r = cpool.tile([128, NSWV], f32r)
            nc.vector.tensor_copy(out=swv_r[:], in_=swv_sb[:])
            ones_c = cpool.tile([128, 16], f32)
            nc.vector.memset(ones_c[:], 1.0)

            seq_sb = cpool.tile([128, NT], i32)
            nc.sync.dma_start(out=seq_sb[:], in_=seq_idx[:])
            cnd_sb = cpool.tile([128, NCT], i32)
            nc.sync.dma_start(out=cnd_sb[:], in_=cand_idx[:])

            ecw1_r = cpool.tile([128, 2, D], f32r)
            ecw2_r = cpool.tile([128, 2, ECO], f32r)
            for k in range(2):
                st = cpool.tile([128, D], f32, tag="wstage")
                nc.sync.dma_start(out=st[:], in_=ecw1[k])
                nc.vector.tensor_copy(out=ecw1_r[:, k, :], in_=st[:])
                st2 = cpool.tile([128, ECO], f32, tag="wstage2")
                nc.sync.dma_start(out=st2[:], in_=ecw2[k])
                nc.vector.tensor_copy(out=ecw2_r[:, k, :], in_=st2[:])
            sdw_r = cpool.tile([64, D], f32r)
            st = cpool.tile([64, D], f32, tag="wstage64")
            nc.sync.dma_start(out=st[:], in_=sdw[:])
            nc.vector.tensor_copy(out=sdw_r[:], in_=st[:])

            wt_dram = dram.tile([BL, 128, 4, S], f32)
            xfin_dram = dram.tile([T, D], f32)

            xpool_cm = tc.tile_pool(name="xpool", bufs=1)
            xpool = xpool_cm.__enter__()
            x_tok = xpool.tile([128, NT, D], f32)

            def fmbv(name, j=0):
                cix = _FMB_COLS[name] + j
                return fmb_sb[:, cix:cix + 1]

            def scv(j, p=128):
                return scal_sb[0:p, j:j + 1]

            # =========== Stage B: gather + embed ===========
            with tc.tile_pool(name="embp", bufs=3) as ep, \
                 tc.tile_pool(name="embps", bufs=2, space="PSUM") as eps_p:
                sideT = ep.tile([64, NT, 128], f32r, tag="sideT")
                for j in range(NT):
                    emb = ep.tile([128, D], f32, tag="emb")
                    nc.gpsimd.indirect_dma_start(
                        out=emb[:], out_offset=None, in_=item_tab[:],
                        in_offset=bass.IndirectOffsetOnAxis(
                            ap=seq_sb[:, j:j + 1], axis=0))
                    sg = ep.tile([128, SIDE], f32, tag="sg")
                    nc.gpsimd.indirect_dma_start(
                        out=sg[:], out_offset=None, in_=side_tab[:],
                        in_offset=bass.IndirectOffsetOnAxis(
                            ap=seq_sb[:, j:j + 1], axis=0))
                    nc.vector.tensor_tensor(out=x_tok[:, j, :], in0=emb[:],
                                            in1=pos_sb[:, j % 4, :], op=ALU.add)
                    tps = eps_p.tile([64, 128], f32, tag="tps")
                    nc.tensor.transpose(out=tps[:], in_=sg[:], identity=ident[:])
                    nc.vector.tensor_copy(out=sideT[:, j, :], in_=tps[:])
                for j in range(NT):
                    mmp = eps_p.tile([128, D], f32, tag="mmp")
                    nc.tensor.matmul(out=mmp[:], lhsT=sideT[:, j, :], rhs=sdw_r[:],
                                     start=True, stop=True)
                    nc.vector.tensor_tensor(out=x_tok[:, j, :], in0=x_tok[:, j, :],
                                            in1=mmp[:], op=ALU.add)
                    if "sideb" not in zflags:
                        nc.vector.tensor_tensor(out=x_tok[:, j, :],
                                                in0=x_tok[:, j, :],
                                                in1=bcc_sb[:, _BCC["sideb"], :],
                                                op=ALU.add)
            if dbg:
                nc.sync.dma_start(out=y_dx0[:], in_=x_tok[:, 0, :])

            # =========== Stage E: eco(x) -> gamma -> attention weights ===========
            gpool_cm = tc.tile_pool(name="gpool", bufs=1)
            gpool = gpool_cm.__enter__()
            gamma_row = gpool.tile([1, T], f32)
            with tc.tile_pool(name="ecop", bufs=2) as ecp, \
                 tc.tile_pool(name="ecops", bufs=1, space="PSUM") as ecps:
                for b in range(BL):
                    xTb = ecp.tile([128, 2, S], f32r, tag="xTb")
                    for tb in range(4):
                        for f in range(2):
                            tp = ecps.tile([128, 128], f32, tag="tp")
                            nc.tensor.transpose(
                                out=tp[:],
                                in_=x_tok[:, b * 4 + tb, f * 128:(f + 1) * 128],
                                identity=ident[:])
                            nc.scalar.copy(
                                out=xTb[:, f, tb * 128:(tb + 1) * 128], in_=tp[:])
                    h1 = ecp.tile([128, 2, S], f32r, tag="h1")
                    for f in range(2):
                        pm = ecps.tile([128, S], f32, tag="pm")
                        for k in range(2):
                            nc.tensor.matmul(
                                out=pm[:], lhsT=ecw1_r[:, k, f * 128:(f + 1) * 128],
                                rhs=xTb[:, k, :], start=(k == 0), stop=(k == 1))
                        nc.scalar.activation(out=h1[:, f, :], in_=pm[:], func=AF.Gelu,
                                             bias=fmbv("ecob1", f))
                    pm2 = ecps.tile([128, S], f32, tag="pm2")
                    for k in range(2):
                        nc.tensor.matmul(out=pm2[:], lhsT=ecw2_r[:, k, :],
                                         rhs=h1[:, k, :], start=(k == 0), stop=(k == 1))
                    ecoT = ecp.tile([128, S], f32r, tag="ecoT")
                    nc.vector.tensor_scalar(out=ecoT[:], in0=pm2[:],
                                            scalar1=fmbv("ecob2"), scalar2=None,
                                            op0=ALU.add)
                    pg = ecps.tile([1, S], f32, tag="pg")
                    nc.tensor.matmul(out=pg[:],
                                     lhsT=swv_r[:, _SWV["gamW"]:_SWV["gamW"] + 1],
                                     rhs=ecoT[:], start=True, stop=True)
                    nc.scalar.activation(out=gamma_row[0:1, b * S:(b + 1) * S],
                                         in_=pg[:], func=AF.Sigmoid,
                                         bias=scv(0, 1))
            if dbg:
                nc.sync.dma_start(out=y_dgam[:], in_=gamma_row[:])

            with tc.tile_pool(name="wtp", bufs=2) as wtp, \
                 tc.tile_pool(name="dtp", bufs=1) as dtp:
                deltaT = dtp.tile([128, 4, S], f32)
                for kt in range(4):
                    nc.sync.dma_start(out=deltaT[:, kt, :], in_=delta4[kt])
                for b in range(BL):
                    gbs = wtp.tile([128, S], f32, tag="gbs")
                    nc.gpsimd.partition_broadcast(
                        out_ap=gbs[:], in_ap=gamma_row[0:1, b * S:(b + 1) * S])
                    ut = wtp.tile([128, 4, S], f32, tag="ut")
                    nc.vector.tensor_tensor(
                        out=ut[:, :, :],
                        in0=gbs[:].unsqueeze(1).to_broadcast([128, 4, S]),
                        in1=deltaT[:, :, :], op=ALU.mult)
                    # w = exp(-ln(u + 1 + 1e-8)) = 1/(1 + gamma*delta + 1e-8)
                    nc.scalar.activation(out=ut[:], in_=ut[:], func=AF.Ln,
                                         bias=scv(5))
                    wtb = wtp.tile([128, 4, S], f32, tag="wtb")
                    nc.scalar.activation(out=wtb[:], in_=ut[:], func=AF.Exp,
                                         scale=-1.0)
                    nc.sync.dma_start(out=wt_dram[b], in_=wtb[:])
            gpool_cm.__exit__(None, None, None)

            # =========== Layers ===========
            def ln_phase(g_ix, b_ix, pool, spill=False):
                aggs = pool.tile([128, NT, 2], f32, tag="lnagg")
                for j in range(NT):
                    stt = pool.tile([128, 6], f32, tag="lnst")
                    nc.vector.bn_stats(out=stt[:], in_=x_tok[:, j, :])
                    nc.vector.bn_aggr(out=aggs[:, j, :], in_=stt[:])
                sd32 = pool.tile([128, NT, 1], f32, tag="lnsd")
                nc.scalar.activation(out=sd32[:], in_=aggs[:, :, 1:2],
                                     func=AF.Sqrt, bias=scv(3))
                rs32 = pool.tile([128, NT, 1], f32, tag="lnrs")
                nc.vector.reciprocal(out=rs32[:], in_=sd32[:])
                for j in range(NT):
                    nc.vector.tensor_scalar(out=x_tok[:, j, :],
                                            in0=x_tok[:, j, :],
                                            scalar1=aggs[:, j, 0:1],
                                            scalar2=rs32[:, j, 0:1],
                                            op0=ALU.subtract, op1=ALU.mult)
                    if "ln_affine" not in zflags:
                        nc.vector.tensor_tensor(out=x_tok[:, j, :],
                                                in0=x_tok[:, j, :],
                                                in1=bcc_sb[:, g_ix, :],
                                                op=ALU.mult)
                        nc.vector.tensor_tensor(out=x_tok[:, j, :],
                                                in0=x_tok[:, j, :],
                                                in1=bcc_sb[:, b_ix, :],
                                                op=ALU.add)
                    if spill:
                        nc.sync.dma_start(
                            out=xfin_dram[j * 128:(j + 1) * 128, :],
                            in_=x_tok[:, j, :])

            with tc.tile_pool(name="lw", bufs=1) as lw:
                for l in range(L):
                    qw_r = lw.tile([128, 2, D], f32r, tag="qw_r")
                    kw_r = lw.tile([128, 2, D], f32r, tag="kw_r")
                    vw_r = lw.tile([128, 2, D], f32r, tag="vw_r")
                    ow_r = lw.tile([128, 2, D], f32r, tag="ow_r")
                    f1_r = lw.tile([128, 2, 4 * D], f32r, tag="f1_r")
                    f2_r = lw.tile([128, 8, D], f32r, tag="f2_r")
                    for k in range(2):
                        for src, dst in [(qw, qw_r), (kw, kw_r), (vw, vw_r),
                                         (ow, ow_r)]:
                            stg = lw.tile([128, D], f32, tag="lstg")
                            nc.sync.dma_start(out=stg[:], in_=src[l, k])
                            nc.vector.tensor_copy(out=dst[:, k, :], in_=stg[:])
                        stg1 = lw.tile([128, 4 * D], f32, tag="lstg1")
                        nc.sync.dma_start(out=stg1[:], in_=f1w[l, k])
                        nc.vector.tensor_copy(out=f1_r[:, k, :], in_=stg1[:])
                    for k in range(8):
                        stg = lw.tile([128, D], f32, tag="lstg")
                        nc.sync.dma_start(out=stg[:], in_=f2w[l, k])
                        nc.vector.tensor_copy(out=f2_r[:, k, :], in_=stg[:])

                    # ---- phase A: attention (ACT: Exp) ----
                    with tc.tile_pool(name=f"at{l}", bufs=2) as ap, \
                         tc.tile_pool(name=f"a1_{l}", bufs=2) as a1, \
                         tc.tile_pool(name=f"atp{l}", bufs=1, space="PSUM") as aps, \
                         tc.tile_pool(name=f"atp2{l}", bufs=2, space="PSUM") as aps2:
                        for b in range(BL):
                            xTb = ap.tile([128, 2, S], f32r, tag="xTb")
                            for tb in range(4):
                                for f in range(2):
                                    tp = aps.tile([128, 512], f32, tag="pq")
                                    nc.tensor.transpose(
                                        out=tp[:, 0:128],
                                        in_=x_tok[:, b * 4 + tb,
                                                  f * 128:(f + 1) * 128],
                                        identity=ident[:])
                                    nc.scalar.copy(
                                        out=xTb[:, f, tb * 128:(tb + 1) * 128],
                                        in_=tp[:, 0:128])
                            qT = ap.tile([128, 2, S], f32r, tag="qT")
                            kT = ap.tile([128, 2, S], f32r, tag="kT")
                            for f in range(2):
                                pq = aps.tile([128, S], f32, tag="pq")
                                for k in range(2):
                                    nc.tensor.matmul(
                                        out=pq[:],
                                        lhsT=qw_r[:, k, f * 128:(f + 1) * 128],
                                        rhs=xTb[:, k, :], start=(k == 0),
                                        stop=(k == 1))
                                if "qb" in zflags:
                                    nc.scalar.copy(out=qT[:, f, :], in_=pq[:])
                                else:
                                    nc.vector.tensor_scalar(
                                        out=qT[:, f, :], in0=pq[:],
                                        scalar1=fmbv(f"qb{l}", f), scalar2=None,
                                        op0=ALU.add)
                                pk = aps.tile([128, S], f32, tag="pq")
                                for k in range(2):
                                    nc.tensor.matmul(
                                        out=pk[:],
                                        lhsT=kw_r[:, k, f * 128:(f + 1) * 128],
                                        rhs=xTb[:, k, :], start=(k == 0),
                                        stop=(k == 1))
                                if "kb" in zflags:
                                    nc.scalar.copy(out=kT[:, f, :], in_=pk[:])
                                else:
                                    nc.vector.tensor_scalar(
                                        out=kT[:, f, :], in0=pk[:],
                                        scalar1=fmbv(f"kb{l}", f), scalar2=None,
                                        op0=ALU.add)
                            vb_t = ap.tile([128, 4, 4 * 65], f32r, tag="vb_t")
                            for tb in range(4):
                                pv = aps.tile([128, D], f32, tag="pv")
                                for k in range(2):
                                    nc.tensor.matmul(
                                        out=pv[:],
                                        lhsT=xTb[:, k, tb * 128:(tb + 1) * 128],
                                        rhs=vw_r[:, k, :], start=(k == 0),
                                        stop=(k == 1))
                                dst = vb_t[:, tb, :].rearrange(
                                    "p (h c) -> p h c", h=4)[:, :, 0:64]
                                if "vb" in zflags:
                                    nc.vector.tensor_copy(
                                        out=dst,
                                        in_=pv[:].rearrange("p (h c) -> p h c",
                                                            h=4))
                                else:
                                    nc.vector.tensor_tensor(
                                        out=dst,
                                        in0=pv[:].rearrange("p (h c) -> p h c",
                                                            h=4),
                                        in1=bcc_sb[:, _BCC[f"vb{l}"], :].rearrange(
                                            "p (h c) -> p h c", h=4),
                                        op=ALU.add)
                                nc.vector.tensor_copy(
                                    out=vb_t[:, tb, :].rearrange(
                                        "p (h c) -> p h c", h=4)[:, :, 64:65],
                                    in_=ones_c[:, 0:4].unsqueeze(2))
                            wtb = ap.tile([128, 4, S], f32, tag="wtb")
                            nc.sync.dma_start(out=wtb[:], in_=wt_dram[b])
                            oTb = ap.tile([128, 2, S], f32r, tag="oTb")
                            for h in range(4):
                                hf, hp = h // 2, (h % 2) * 64
                                scp = aps.tile([128, 4, S], f32, tag="scp")
                                for kt in range(4):
                                    nc.tensor.matmul(
                                        out=scp[:, kt, :],
                                        lhsT=kT[hp:hp + 64, hf,
                                                kt * 128:(kt + 1) * 128],
                                        rhs=qT[hp:hp + 64, hf, :],
                                        start=True, stop=True)
                                eU = a1.tile([128, 4, S], f32, tag="eU")
                                nc.scalar.activation(out=eU[:], in_=scp[:],
                                                     func=AF.Exp)
                                aU = a1.tile([128, 4, S], f32r, tag="aU")
                                nc.vector.tensor_tensor(out=aU[:], in0=eU[:],
                                                        in1=wtb[:], op=ALU.mult)
                                avp = aps2.tile([65, S], f32, tag="avp")
                                for kt in range(4):
                                    nc.tensor.matmul(
                                        out=avp[:],
                                        lhsT=vb_t[:, kt, h * 65:(h + 1) * 65],
                                        rhs=aU[:, kt, :], start=(kt == 0),
                                        stop=(kt == 3))
                                if dbg and l == 0 and b == 0 and h == 0:
                                    dat = a1.tile([128, S], f32, tag="dat")
                                    nc.vector.memset(dat[:], 0.0)
                                    nc.vector.tensor_copy(out=dat[0:65, :],
                                                          in_=avp[:])
                                    nc.sync.dma_start(out=y_dattn[:], in_=dat[:])
                                rr = a1.tile([1, S], f32, tag="rr")
                                nc.vector.reciprocal(out=rr[:], in_=avp[64:65, :])
                                rbs = a1.tile([64, S], f32, tag="rbs")
                                nc.gpsimd.partition_broadcast(out_ap=rbs[:],
                                                              in_ap=rr[:])
                                nc.vector.tensor_tensor(
                                    out=oTb[hp:hp + 64, hf, :], in0=avp[0:64, :],
                                    in1=rbs[:], op=ALU.mult)
                            for tb in range(4):
                                po = aps.tile([128, D], f32, tag="pv")
                                for f in range(2):
                                    nc.tensor.matmul(
                                        out=po[:],
                                        lhsT=oTb[:, f, tb * 128:(tb + 1) * 128],
                                        rhs=ow_r[:, f, :], start=(f == 0),
                                        stop=(f == 1))
                                j = b * 4 + tb
                                nc.vector.tensor_tensor(
                                    out=x_tok[:, j, :], in0=x_tok[:, j, :],
                                    in1=po[:], op=ALU.add)
                                if "ob" not in zflags:
                                    nc.vector.tensor_tensor(
                                        out=x_tok[:, j, :], in0=x_tok[:, j, :],
                                        in1=bcc_sb[:, _BCC[f"ob{l}"], :],
                                        op=ALU.add)

                    # ---- phase B: LN1 (ACT: Sqrt) ----
                    with tc.tile_pool(name=f"l1_{l}", bufs=2) as lnp:
                        ln_phase(_BCC[f"n1g{l}"], _BCC[f"n1b{l}"], lnp)

                    # ---- phase C: FF (ACT: Gelu) ----
                    with tc.tile_pool(name=f"ff{l}", bufs=2) as fp, \
                         tc.tile_pool(name=f"ffp{l}", bufs=1, space="PSUM") as fps:
                        for b in range(BL):
                            xTb = fp.tile([128, 2, S], f32r, tag="xTb")
                            for tb in range(4):
                                for f in range(2):
                                    tp = fps.tile([128, 128], f32, tag="tp")
                                    nc.tensor.transpose(
                                        out=tp[:],
                                        in_=x_tok[:, b * 4 + tb,
                                                  f * 128:(f + 1) * 128],
                                        identity=ident[:])
                                    nc.scalar.copy(
                                        out=xTb[:, f, tb * 128:(tb + 1) * 128],
                                        in_=tp[:])
                            hT = fp.tile([128, 8, S], f32r, tag="hT")
                            for fo in range(8):
                                pf = fps.tile([128, S], f32, tag="pf")
                                for k in range(2):
                                    nc.tensor.matmul(
                                        out=pf[:],
                                        lhsT=f1_r[:, k, fo * 128:(fo + 1) * 128],
                                        rhs=xTb[:, k, :], start=(k == 0),
                                        stop=(k == 1))
                                nc.scalar.activation(out=hT[:, fo, :], in_=pf[:],
                                                     func=AF.Gelu,
                                                     bias=fmbv(f"ffb1_{l}", fo))
                            for tb in range(4):
                                p2 = fps.tile([128, D], f32, tag="p2")
                                for k in range(8):
                                    nc.tensor.matmul(
                                        out=p2[:],
                                        lhsT=hT[:, k, tb * 128:(tb + 1) * 128],
                                        rhs=f2_r[:, k, :], start=(k == 0),
                                        stop=(k == 7))
                                j = b * 4 + tb
                                nc.vector.tensor_tensor(
                                    out=x_tok[:, j, :], in0=x_tok[:, j, :],
                                    in1=p2[:], op=ALU.add)
                                if "ffb2" not in zflags:
                                    nc.vector.tensor_tensor(
                                        out=x_tok[:, j, :], in0=x_tok[:, j, :],
                                        in1=bcc_sb[:, _BCC[f"ffb2_{l}"], :],
                                        op=ALU.add)

                    # ---- phase D: LN2 (ACT: Sqrt) ----
                    with tc.tile_pool(name=f"l2_{l}", bufs=2) as lnp:
                        ln_phase(_BCC[f"n2g{l}"], _BCC[f"n2b{l}"], lnp)
                    if dbg and l == 0:
                        nc.sync.dma_start(out=y_dl0[:], in_=x_tok[:, 0, :])

                # final LN + spill to DRAM
                with tc.tile_pool(name="lnf", bufs=2) as lnp:
                    ln_phase(_BCC["fng"], _BCC["fnb"], lnp, spill=True)
                if dbg:
                    nc.sync.dma_start(out=y_dxf[:], in_=x_tok[:, 0, :])
            xpool_cm.__exit__(None, None, None)

            # =========== Head ===========
            with tc.tile_pool(name="hd", bufs=1) as hd, \
                 tc.tile_pool(name="hdl", bufs=3) as hdl, \
                 tc.tile_pool(name="hps", bufs=2, space="PSUM") as hps, \
                 tc.tile_pool(name="hpl", bufs=1, space="PSUM") as hpl:

                def loadw(src, kdim, width, tg):
                    t = hd.tile([128, kdim, width], f32r, tag=tg)
                    for k in range(kdim):
                        stg = hdl.tile([128, width], f32, tag="hstg")
                        nc.sync.dma_start(out=stg[:], in_=src[k])
                        nc.vector.tensor_copy(out=t[:, k, :], in_=stg[:])
                    return t

                wi_r = loadw(wiw, 2, D, "wi_r")
                wi2_r = loadw(wi2w, 2, D, "wi2_r")
                smw1_r = loadw(smw1, 2, D, "smw1_r")
                smw2_r = loadw(smw2, 2, SEM, "smw2_r")
                wew_r = hd.tile([128, D], f32r, tag="wew_r")
                stg = hdl.tile([128, D], f32, tag="hstg")
                nc.sync.dma_start(out=stg[:], in_=wew[:])
                nc.vector.tensor_copy(out=wew_r[:], in_=stg[:])
                wsw_r = hd.tile([128, D], f32r, tag="wsw_r")
                stg = hdl.tile([128, D], f32, tag="hstg")
                nc.sync.dma_start(out=stg[:], in_=wsw[:])
                nc.vector.tensor_copy(out=wsw_r[:], in_=stg[:])
                pwp_sb = hd.tile([128, 2, P], f32)
                nc.sync.dma_start(out=pwp_sb[:], in_=pwp[:])
                prior_sb = hd.tile([8, CP, P], f32)
                nc.sync.dma_start(out=prior_sb[:], in_=prior_bc[:])

                li = hdl.tile([8, 1], i32, tag="li")
                nc.sync.dma_start(out=li[:], in_=last_idx[:])
                lastg = hdl.tile([8, D], f32, tag="lastg")
                nc.gpsimd.indirect_dma_start(
                    out=lastg[:], out_offset=None, in_=xfin_dram[:],
                    in_offset=bass.IndirectOffsetOnAxis(ap=li[:, :1], axis=0))
                lastT = hd.tile([128, 2, 8], f32r, tag="lastT")
                for f in range(2):
                    tp = hps.tile([128, 8], f32, tag="hmm")
                    nc.tensor.transpose(out=tp[:],
                                        in_=lastg[:, f * 128:(f + 1) * 128],
                                        identity=ident[0:8, 0:8])
                    nc.vector.tensor_copy(out=lastT[:, f, :], in_=tp[:])

                def mlp2(w1_r, b1n, w2_r, b2n, width2, tg):
                    h1 = hd.tile([128, 2, 8], f32r, tag=tg + "h")
                    for f in range(2):
                        pm = hps.tile([128, 8], f32, tag="hmm")
                        for k in range(2):
                            nc.tensor.matmul(
                                out=pm[:], lhsT=w1_r[:, k, f * 128:(f + 1) * 128],
                                rhs=lastT[:, k, :], start=(k == 0), stop=(k == 1))
                        nc.scalar.activation(out=h1[:, f, :], in_=pm[:],
                                             func=AF.Gelu, bias=fmbv(b1n, f))
                    pm2 = hps.tile([128, 8], f32, tag="hmm")
                    for k in range(2):
                        nc.tensor.matmul(out=pm2[:], lhsT=w2_r[:, k, 0:width2],
                                         rhs=h1[:, k, :], start=(k == 0),
                                         stop=(k == 1))
                    out = hd.tile([128, 8], f32r, tag=tg)
                    nc.vector.tensor_scalar(out=out[0:width2, :],
                                            in0=pm2[0:width2, :],
                                            scalar1=fmbv(b2n)[0:width2, :],
                                            scalar2=None, op0=ALU.add)
                    return out

                ecoS = mlp2(ecw1_r, "ecob1", ecw2_r, "ecob2", ECO, "ecoS")
                semS = mlp2(smw1_r, "semb1", smw2_r, "semb2", SEM, "semS")
                if dbg:
                    dcs = hdl.tile([128, 8], f32, tag="dcs")
                    nc.vector.tensor_copy(out=dcs[:], in_=ecoS[:])
                    nc.sync.dma_start(out=y_decs[:], in_=dcs[:])

                prp = hd.tile([128, NCT], f32, tag="prp")
                icT = hd.tile([128, 2, TCD], f32r, tag="icT")
                for j in range(NCT):
                    ce = hdl.tile([128, D], f32, tag="ce")
                    nc.gpsimd.indirect_dma_start(
                        out=ce[:], out_offset=None, in_=item_tab[:],
                        in_offset=bass.IndirectOffsetOnAxis(
                            ap=cnd_sb[:, j:j + 1], axis=0))
                    cs = hdl.tile([128, SIDE], f32, tag="cs")
                    nc.gpsimd.indirect_dma_start(
                        out=cs[:], out_offset=None, in_=side_tab[:],
                        in_offset=bass.IndirectOffsetOnAxis(
                            ap=cnd_sb[:, j:j + 1], axis=0))
                    pp = hdl.tile([128, 1], f32, tag="pp")
                    nc.gpsimd.indirect_dma_start(
                        out=pp[:], out_offset=None, in_=prop_tab[:],
                        in_offset=bass.IndirectOffsetOnAxis(
                            ap=cnd_sb[:, j:j + 1], axis=0))
                    nc.vector.tensor_copy(out=prp[:, j:j + 1], in_=pp[:])
                    tps = hps.tile([64, 128], f32, tag="hmm")
                    nc.tensor.transpose(out=tps[:], in_=cs[:], identity=ident[:])
                    csT = hdl.tile([64, 128], f32r, tag="csT")
                    nc.vector.tensor_copy(out=csT[:], in_=tps[:])
                    pm = hps.tile([128, D], f32, tag="hmm")
                    nc.tensor.matmul(out=pm[:], lhsT=csT[:], rhs=sdw_r[:],
                                     start=True, stop=True)
                    ic = hdl.tile([128, D], f32, tag="ic")
                    nc.vector.tensor_tensor(out=ic[:], in0=ce[:],
                                            in1=pm[:], op=ALU.add)
                    if "sideb" not in zflags:
                        nc.vector.tensor_tensor(out=ic[:], in0=ic[:],
                                                in1=bcc_sb[:, _BCC["sideb"], :],
                                                op=ALU.add)
                    for f in range(2):
                        tp = hps.tile([128, 128], f32, tag="hmm")
                        nc.tensor.transpose(out=tp[:],
                                            in_=ic[:, f * 128:(f + 1) * 128],
                                            identity=ident[:])
                        nc.vector.tensor_copy(
                            out=icT[:, f, j * 128:(j + 1) * 128], in_=tp[:])

                icWT = hd.tile([128, 2, TCD], f32, tag="icWT")
                eWeT = hd.tile([128, 2, 8], f32, tag="eWeT")
                swbT = hd.tile([128, 2, 8], f32, tag="swbT")
                for f in range(2):
                    for ch in range(4):
                        pm = hps.tile([128, S], f32, tag="hmm")
                        for k in range(2):
                            nc.tensor.matmul(
                                out=pm[:], lhsT=wi_r[:, k, f * 128:(f + 1) * 128],
                                rhs=icT[:, k, ch * S:(ch + 1) * S],
                                start=(k == 0), stop=(k == 1))
                        nc.vector.tensor_scalar(
                            out=icWT[:, f, ch * S:(ch + 1) * S], in0=pm[:],
                            scalar1=fmbv("dob", f), scalar2=None, op0=ALU.add)
                    pe = hps.tile([128, 8], f32, tag="hmm")
                    nc.tensor.matmul(out=pe[:],
                                     lhsT=wew_r[:, f * 128:(f + 1) * 128],
                                     rhs=ecoS[0:ECO, :], start=True, stop=True)
                    nc.vector.tensor_copy(out=eWeT[:, f, :], in_=pe[:])
                    ps_ = hps.tile([128, 8], f32, tag="hmm")
                    nc.tensor.matmul(out=ps_[:],
                                     lhsT=wsw_r[:, f * 128:(f + 1) * 128],
                                     rhs=semS[0:SEM, :], start=True, stop=True)
                    nc.vector.tensor_scalar(out=swbT[:, f, :], in0=ps_[:],
                                            scalar1=fmbv("shb1", f), scalar2=None,
                                            op0=ALU.add)

                aT = hd.tile([128, 2, 8 * P], f32, tag="aT")
                for f in range(2):
                    nc.vector.tensor_tensor(
                        out=aT[:, f, :].rearrange("d (b p) -> d b p", b=8),
                        in0=eWeT[:, f, :].unsqueeze(2).to_broadcast([128, 8, P]),
                        in1=pwp_sb[:, f, :].unsqueeze(1).to_broadcast([128, 8, P]),
                        op=ALU.add)

                hsh = hd.tile([128, 2, TCD], f32r, tag="hsh")
                for f in range(2):
                    for b in range(BL):
                        pm2 = hps.tile([128, CP], f32, tag="hmm")
                        for k in range(2):
                            nc.tensor.matmul(
                                out=pm2[:], lhsT=wi2_r[:, k, f * 128:(f + 1) * 128],
                                rhs=icT[:, k, b * CP:(b + 1) * CP],
                                start=(k == 0), stop=(k == 1))
                        nc.scalar.activation(out=hsh[:, f, b * CP:(b + 1) * CP],
                                             in_=pm2[:],
                                             func=AF.Gelu, bias=swbT[:, f, b:b + 1])

                lg8 = hd.tile([8, CP, P], f32, tag="lg8")
                sh8 = hd.tile([8, CP], f32, tag="sh8")
                for b in range(BL):
                    preT = hd.tile([128, 2, CP * P], f32, tag="preT")
                    for f in range(2):
                        nc.vector.tensor_tensor(
                            out=preT[:, f, :].rearrange("d (c p) -> d c p", c=CP),
                            in0=icWT[:, f, b * CP:(b + 1) * CP].unsqueeze(2)
                                .to_broadcast([128, CP, P]),
                            in1=aT[:, f, b * P:(b + 1) * P].unsqueeze(1)
                                .to_broadcast([128, CP, P]),
                            op=ALU.add)
                    tnh = hd.tile([128, 2, CP * P], f32r, tag="tnh")
                    nc.scalar.activation(out=tnh[:], in_=preT[:], func=AF.Tanh)
                    lgp = hpl.tile([1, CP * P], f32, tag="lgp")
                    for ch in range(4):
                        for f in range(2):
                            nc.tensor.matmul(
                                out=lgp[0:1, ch * S:(ch + 1) * S],
                                lhsT=swv_r[:, _SWV[f"doOutW_{f}"]:
                                           _SWV[f"doOutW_{f}"] + 1],
                                rhs=tnh[:, f, ch * S:(ch + 1) * S],
                                start=(f == 0), stop=(f == 1))
                    lrow = hd.tile([1, CP * P], f32, tag="lrow")
                    nc.vector.tensor_copy(out=lrow[:], in_=lgp[:])
                    nc.sync.dma_start(
                        out=lg8[b:b + 1, :, :],
                        in_=lrow[:].rearrange("a (c p) -> a c p", c=CP))
                    shp = hps.tile([1, CP], f32, tag="hmm")
                    for f in range(2):
                        nc.tensor.matmul(
                            out=shp[:],
                            lhsT=swv_r[:, _SWV[f"shW2_{f}"]:_SWV[f"shW2_{f}"] + 1],
                            rhs=hsh[:, f, b * CP:(b + 1) * CP],
                            start=(f == 0), stop=(f == 1))
                    srow = hdl.tile([1, CP], f32, tag="srow")
                    nc.vector.tensor_scalar(out=srow[:], in0=shp[:],
                                            scalar1=scv(7, 1), scalar2=None,
                                            op0=ALU.add)
                    nc.sync.dma_start(out=sh8[b:b + 1, :], in_=srow[:])

                do8 = hd.tile([8, CP], f32, tag="do8")
                lgw = hd.tile([8, CP, P], f32, tag="lgw")
                nc.vector.tensor_tensor(out=lgw[:], in0=lg8[:], in1=prior_sb[:],
                                        op=ALU.mult)
                nc.vector.tensor_reduce(out=do8[:], in_=lgw[:], axis=AX.X,
                                        op=ALU.add)
                nc.vector.tensor_scalar(out=do8[:], in0=do8[:], scalar1=scv(6, 8),
                                        scalar2=None, op0=ALU.add)

                mabs = hdl.tile([8, 1], f32, tag="mabs")
                nc.vector.tensor_reduce(out=mabs[:], in_=do8[:, 0:C], axis=AX.X,
                                        op=ALU.add)
                nc.vector.tensor_scalar(out=mabs[:], in0=mabs[:], scalar1=1.0 / C,
                                        scalar2=None, op0=ALU.mult)

                br_p = hps.tile([1, 8], f32, tag="hmm")
                nc.tensor.matmul(out=br_p[:],
                                 lhsT=swv_r[:, _SWV["refW"]:_SWV["refW"] + 1],
                                 rhs=ecoS[0:ECO, :], start=True, stop=True)
                bref_row = hdl.tile([1, 8], f32, tag="bref_row")
                nc.vector.tensor_scalar(out=bref_row[:], in0=br_p[:],
                                        scalar1=scv(1, 1), scalar2=None,
                                        op0=ALU.add)
                et_p = hps.tile([1, 8], f32, tag="hmm")
                nc.tensor.matmul(out=et_p[:],
                                 lhsT=swv_r[:, _SWV["etaW"]:_SWV["etaW"] + 1],
                                 rhs=ecoS[0:ECO, :], start=True, stop=True)
                # mabs -> row layout
                mb_p = hps.tile([1, 8], f32, tag="hmm")
                nc.tensor.transpose(out=mb_p[:], in_=mabs[:],
                                    identity=ident[0:8, 0:8])
                mabs_row = hdl.tile([1, 8], f32, tag="mabs_row")
                nc.vector.tensor_copy(out=mabs_row[:], in_=mb_p[:])
                # eta = sigmoid(etaraw + etaW_last*mabs + etab)  (row layout)
                etav = hdl.tile([1, 8], f32, tag="etav")
                nc.vector.tensor_scalar(out=etav[:], in0=mabs_row[:],
                                        scalar1=scv(8, 1), scalar2=None,
                                        op0=ALU.mult)
                nc.vector.tensor_tensor(out=etav[:], in0=etav[:],
                                        in1=et_p[:], op=ALU.add)
                eta = hdl.tile([1, 8], f32, tag="eta")
                nc.scalar.activation(out=eta[:], in_=etav[:], func=AF.Sigmoid,
                                     bias=scv(2, 1))
                # ref = bref + eta*(mabs - bref)   (row layout)
                dref = hdl.tile([1, 8], f32, tag="dref")
                nc.vector.tensor_tensor(out=dref[:], in0=mabs_row[:],
                                        in1=bref_row[:], op=ALU.subtract)
                nc.vector.tensor_tensor(out=dref[:], in0=dref[:], in1=eta[:],
                                        op=ALU.mult)
                ref_row = hdl.tile([1, 8], f32, tag="ref_row")
                nc.vector.tensor_tensor(out=ref_row[:], in0=bref_row[:],
                                        in1=dref[:], op=ALU.add)
                rf_p = hps.tile([8, 1], f32, tag="hmm")
                nc.tensor.transpose(out=rf_p[:], in_=ref_row[:],
                                    identity=ident[0:1, 0:1])
                ref8 = hdl.tile([8, 1], f32, tag="ref8")
                nc.vector.tensor_copy(out=ref8[:], in_=rf_p[:])
                nc.sync.dma_start(out=y_ref[:], in_=ref8[:, 0:1])

                # prospect value
                du = hd.tile([8, CP], f32, tag="du")
                nc.vector.tensor_scalar(out=du[:], in0=do8[:], scalar1=ref8[:, 0:1],
                                        scalar2=None, op0=ALU.subtract)
                gate = hd.tile([8, CP], f32, tag="gate")
                nc.scalar.activation(out=gate[:], in_=du[:], func=AF.Sigmoid,
                                     scale=1.0 / GTEMP)
                du2l = hd.tile([8, CP], f32, tag="du2l")
                nc.scalar.activation(out=du2l[:], in_=du[:], func=AF.Square)
                nc.scalar.activation(out=du2l[:], in_=du2l[:], func=AF.Ln,
                                     bias=scv(4, 8))
                gain = hd.tile([8, CP], f32, tag="gain")
                nc.scalar.activation(out=gain[:], in_=du2l[:], func=AF.Exp,
                                     scale=ALPHA / 2.0)
                lossv = hd.tile([8, CP], f32, tag="lossv")
                nc.scalar.activation(out=lossv[:], in_=du2l[:], func=AF.Exp,
                                     scale=BETA / 2.0)
                longs = hd.tile([8, CP], f32, tag="longs")
                t2 = hd.tile([8, CP], f32, tag="t2")
                nc.vector.tensor_tensor(out=longs[:], in0=gate[:], in1=gain[:],
                                        op=ALU.mult)
                nc.vector.tensor_tensor(out=t2[:], in0=gate[:], in1=lossv[:],
                                        op=ALU.mult)
                nc.vector.tensor_tensor(out=t2[:], in0=t2[:], in1=lossv[:],
                                        op=ALU.subtract)
                nc.vector.tensor_scalar(out=t2[:], in0=t2[:], scalar1=LAM,
                                        scalar2=None, op0=ALU.mult)
                nc.vector.tensor_tensor(out=longs[:], in0=longs[:], in1=t2[:],
                                        op=ALU.add)
                tot8 = hd.tile([8, CP], f32, tag="tot8")
                nc.vector.tensor_scalar(out=tot8[:], in0=longs[:],
                                        scalar1=scv(9, 8), scalar2=None,
                                        op0=ALU.mult)
                nc.vector.tensor_tensor(out=tot8[:], in0=tot8[:], in1=sh8[:],
                                        op=ALU.add)

                # w: transpose prop [128,16] -> [8,256]
                prT_p = hps.tile([16, 128], f32, tag="hmm")
                nc.tensor.transpose(out=prT_p[:], in_=prp[:], identity=ident[:])
                prT = hdl.tile([16, 128], f32, tag="prT")
                nc.vector.tensor_copy(out=prT[:], in_=prT_p[:])
                prp8 = hd.tile([8, 2, 128], f32, tag="prp8")
                for b in range(8):
                    for hi in range(2):
                        nc.sync.dma_start(
                            out=prp8[b:b + 1, hi, :],
                            in_=prT[2 * b + hi:2 * b + hi + 1, :])
                wv = hd.tile([8, CP], f32, tag="wv")
                nc.vector.tensor_scalar(out=wv[:], in0=prp8[:].rearrange(
                    "b h c -> b (h c)"), scalar1=1e-6, scalar2=None, op0=ALU.max)
                nc.vector.reciprocal(out=wv[:], in_=wv[:])
                nc.vector.tensor_scalar(out=wv[:], in0=wv[:], scalar1=TAU,
                                        scalar2=None, op0=ALU.min)
                wm = hdl.tile([8, 1], f32, tag="wm")
                nc.vector.tensor_reduce(out=wm[:], in_=wv[:, 0:C], axis=AX.X,
                                        op=ALU.add)
                nc.vector.tensor_scalar(out=wm[:], in0=wm[:], scalar1=1.0 / C,
                                        scalar2=1e-8, op0=ALU.mult, op1=ALU.add)
                nc.vector.reciprocal(out=wm[:], in_=wm[:])
                w8 = hd.tile([8, CP], f32, tag="w8")
                nc.vector.tensor_scalar(out=w8[:], in0=wv[:], scalar1=wm[:, 0:1],
                                        scalar2=None, op0=ALU.mult)

                # ortho partials
                prod = hd.tile([128, 8], f32r, tag="prod")
                nc.vector.tensor_tensor(out=prod[:], in0=semS[:], in1=ecoS[:],
                                        op=ALU.mult)
                os_p = hps.tile([1, 8], f32, tag="hmm")
                nc.tensor.matmul(out=os_p[:],
                                 lhsT=swv_r[:, _SWV["ones"]:_SWV["ones"] + 1],
                                 rhs=prod[:], start=True, stop=True)
                osq = hdl.tile([1, 8], f32, tag="osq")
                nc.scalar.activation(out=osq[:], in_=os_p[:], func=AF.Square)
                nc.sync.dma_start(out=y_osq[:], in_=osq[0:1, :])

                nc.sync.dma_start(out=y_total[:], in_=tot8[:, 0:C])
                nc.sync.dma_start(out=y_short[:], in_=sh8[:, 0:C])
                nc.sync.dma_start(out=y_long[:], in_=longs[:, 0:C])
                nc.sync.dma_start(out=y_do[:], in_=do8[:, 0:C])
                nc.sync.dma_start(out=y_w[:], in_=w8[:, 0:C])

    nc.finalize()
    return nc


def _np_softmax(x):
    e = np.exp(x - x.max())
    return e / e.sum()


def _prep_host(inputs):
    """Shared (core-independent) host-side tensor prep."""
    g = {k: np.asarray(v) for k, v in inputs.items()}
    item_tab = np.ascontiguousarray(g["item_table"].astype(np.float32))
    item_tab = item_tab.copy()
    item_tab[0, :] = 0.0
    side_tab = np.ascontiguousarray(g["side_feat"].astype(np.float32))
    prop_tab = np.ascontiguousarray(
        g["prop"].astype(np.float32).reshape(NI + 1, 1))

    pos4 = np.ascontiguousarray(
        g["pos_table"].astype(np.float32).reshape(4, 128, D))

    kk = np.arange(S)[:, None]
    qq = np.arange(S)[None, :]
    dm = np.where(kk <= qq, (qq - kk).astype(np.float32), np.float32(1e9))
    delta4 = np.ascontiguousarray(dm.reshape(4, 128, S).astype(np.float32))

    sc = 1.0 / np.sqrt(DH)
    qw = np.ascontiguousarray(
        (g["qW"] * sc).astype(np.float32).reshape(L, 2, 128, D))
    kw = np.ascontiguousarray(g["kW"].astype(np.float32).reshape(L, 2, 128, D))
    vw = np.ascontiguousarray(g["vW"].astype(np.float32).reshape(L, 2, 128, D))
    ow = np.ascontiguousarray(g["oW"].astype(np.float32).reshape(L, 2, 128, D))
    f1w = np.ascontiguousarray(
        g["ffW1"].astype(np.float32).reshape(L, 2, 128, 4 * D))
    f2w = np.ascontiguousarray(
        g["ffW2"].astype(np.float32).reshape(L, 8, 128, D))
    ecw1 = np.ascontiguousarray(g["ecoW1"].astype(np.float32).reshape(2, 128, D))
    ecw2 = np.ascontiguousarray(
        g["ecoW2"].astype(np.float32).reshape(2, 128, ECO))
    smw1 = np.ascontiguousarray(g["semW1"].astype(np.float32).reshape(2, 128, D))
    smw2 = np.ascontiguousarray(
        g["semW2"].astype(np.float32).reshape(2, 128, SEM))
    sdw = np.ascontiguousarray(g["side_W"].astype(np.float32))
    doW = g["doW"].astype(np.float32)
    wiw = np.ascontiguousarray(doW[:D].reshape(2, 128, D))
    wew = np.ascontiguousarray(doW[D:D + ECO])
    shW1 = g["shW1"].astype(np.float32)
    wsw = np.ascontiguousarray(shW1[:SEM])
    wi2w = np.ascontiguousarray(shW1[SEM:].reshape(2, 128, D))

    swv = np.zeros((128, NSWV), np.float32)
    swv[:, _SWV["gamW"]] = g["gamW"][:, 0]
    swv[:, _SWV["refW"]] = g["refW"][:, 0]
    swv[:, _SWV["etaW"]] = g["etaW"][:ECO, 0]
    shW2 = g["shW2"].astype(np.float32)
    swv[:, _SWV["shW2_0"]] = shW2[:128, 0]
    swv[:, _SWV["shW2_1"]] = shW2[128:, 0]
    doOutW = g["doOutW"].astype(np.float32)
    swv[:, _SWV["doOutW_0"]] = doOutW[:128, 0]
    swv[:, _SWV["doOutW_1"]] = doOutW[128:, 0]
    swv[:, _SWV["ones"]] = 1.0

    fmb = np.zeros((128, NFMB), np.float32)

    def setf(nm, vec):
        n = len(vec) // 128
        for j in range(n):
            fmb[:, _FMB_COLS[nm] + j] = vec[j * 128:(j + 1) * 128]

    setf("qb0", g["qb"][0] * sc); setf("qb1", g["qb"][1] * sc)
    setf("kb0", g["kb"][0]); setf("kb1", g["kb"][1])
    setf("ecob1", g["ecob1"]); setf("ecob2", g["ecob2"])
    setf("semb1", g["semb1"]); setf("semb2", g["semb2"])
    setf("ffb1_0", g["ffb1"][0]); setf("ffb1_1", g["ffb1"][1])
    setf("dob", g["dob"]); setf("shb1", g["shb1"]); setf("sideb", g["side_b"])

    bcc = np.zeros((NBCC, 128, D), np.float32)

    def setb(nm, vec):
        bcc[_BCC[nm]] = np.tile(np.asarray(vec, np.float32)[None, :], (128, 1))

    setb("sideb", g["side_b"])
    for l in range(L):
        setb(f"vb{l}", g["vb"][l]); setb(f"ob{l}", g["ob"][l])
        setb(f"ffb2_{l}", g["ffb2"][l])
        setb(f"n1g{l}", g["n1g"][l]); setb(f"n1b{l}", g["n1b"][l])
        setb(f"n2g{l}", g["n2g"][l]); setb(f"n2b{l}", g["n2b"][l])
    setb("fng", g["fng"]); setb("fnb", g["fnb"])

    prior = _np_softmax(np.asarray(g["plog"], np.float32))
    prior_bc = np.ascontiguousarray(
        np.broadcast_to(prior[None, None, :], (8, CP, P)).astype(np.float32))
    pwpT = (np.asarray(g["proto"], np.float32) @ doW[D + ECO:]).T  # [D, P]
    pwp = np.ascontiguousarray(pwpT.reshape(2, 128, P).transpose(1, 0, 2))

    kappa = float(np.asarray(g["kappa"]))
    kscale = 1.0 / (1.0 + np.log1p(np.exp(kappa)))
    scal = np.zeros((128, NSCAL), np.float32)
    row = [float(g["gamb"][0]), float(g["refb"][0]), float(g["etab"][0]),
           1e-5, 1e-6, 1.0 + 1e-8, float(g["doOutb"][0]), float(g["shb2"][0]),
           float(g["etaW"][ECO, 0]), kscale]
    scal[:] = np.asarray(row, np.float32)[None, :]

    zf = set()
    if (np.all(g["n1g"] == 1) and np.all(g["n2g"] == 1) and np.all(g["fng"] == 1)
            and np.all(g["n1b"] == 0) and np.all(g["n2b"] == 0)
            and np.all(g["fnb"] == 0)):
        zf.add("ln_affine")
    if np.all(g["side_b"] == 0):
        zf.add("sideb")
    if np.all(g["qb"] == 0):
        zf.add("qb")
    if np.all(g["kb"] == 0):
        zf.add("kb")
    if np.all(g["vb"] == 0):
        zf.add("vb")
    if np.all(g["ob"] == 0):
        zf.add("ob")
    if np.all(g["ffb2"] == 0):
        zf.add("ffb2")
    zflags = frozenset(zf)

    shared = dict(item_tab=item_tab, side_tab=side_tab, prop_tab=prop_tab,
                  pos4=pos4, delta4=delta4, qw=qw, kw=kw, vw=vw, ow=ow,
                  f1w=f1w, f2w=f2w, ecw1=ecw1, ecw2=ecw2, smw1=smw1,
                  smw2=smw2, sdw=sdw, wiw=wiw, wew=wew, wi2w=wi2w, wsw=wsw,
                  swv=swv, fmb=fmb, bcc=bcc, prior_bc=prior_bc, pwp=pwp,
                  scal=scal)

    seq = np.asarray(g["seq"]).astype(np.int64)
    cand = np.asarray(g["candidate_items"]).astype(np.int64)
    lens = np.maximum((seq != 0).sum(1), 1)
    in_maps = []
    for c in range(NCORE):
        sl = slice(c * BL, (c + 1) * BL)
        seq_c = seq[sl].reshape(T).astype(np.int32)
        seq_ix = np.ascontiguousarray(seq_c.reshape(NT, 128).T)
        cand_pad = np.zeros((BL, CP), np.int32)
        cand_pad[:, :C] = cand[sl]
        cand_ix = np.ascontiguousarray(
            cand_pad.reshape(TCD).reshape(NCT, 128).T)
        last_ix = (np.arange(BL) * S + lens[sl] - 1).astype(
            np.int32).reshape(8, 1)
        m = dict(shared)
        m["seq_idx"] = seq_ix
        m["cand_idx"] = cand_ix
        m["last_idx"] = np.ascontiguousarray(last_ix)
        in_maps.append(m)
    return in_maps, zflags


def kernel(dbg=False, _res_out=None, **inputs):
    in_maps, zflags = _prep_host(inputs)
    key = (bool(dbg), zflags)
    if key not in _PROG_CACHE:
        _PROG_CACHE[key] = _build_program(dbg=key[0], zflags=zflags)
    nc = _PROG_CACHE[key]
    res = run_bass_kernel_spmd(nc, in_maps, list(range(NCORE)))
    if _res_out is not None:
        _res_out.append(res)
    rs = res.results
    total = np.concatenate([rs[c]["y_total"] for c in range(NCORE)], 0)
    short_s = np.concatenate([rs[c]["y_short"] for c in range(NCORE)], 0)
    long_s = np.concatenate([rs[c]["y_long"] for c in range(NCORE)], 0)
    do_util = np.concatenate([rs[c]["y_do"] for c in range(NCORE)], 0)
    ref = np.concatenate([rs[c]["y_ref"] for c in range(NCORE)], 0)
    w = np.concatenate([rs[c]["y_w"] for c in range(NCORE)], 0)
    osq = np.concatenate([rs[c]["y_osq"] for c in range(NCORE)], 0)
    ortho = np.float32(osq.mean())
    return (total.astype(np.float32), short_s.astype(np.float32),
            long_s.astype(np.float32), do_util.astype(np.float32),
            ref.astype(np.float32), w.astype(np.float32), ortho)
